# revision 7
# baseline (speedup 1.0000x reference)
"""Conv1D-MHSA (sketched linear attention) Trainium2 kernel, v2.

Math (per batch b, head h; head h -> core h):
    q = conv1d_K3(x_pad, q_w) ; k likewise ; v = conv1d_K1(x, v_w)
    phi_q = tanh((q^T g1_q)*(q^T g2_q)/sqrt(R))  (phi_k likewise; the sqrt(R)
    post-scales cancel between numerator and denominator, eps -> ~0 dropped)
    out_h = diag(1/(phi_q^T s_k)) . phi_q^T (M~ pw),  M~ = phi_k^T v,
    s_k = colsum(phi_k);  host sums the 8 per-head partials + proj_b.

Precision: the denominator path (conv, sketches, phi_q, s_k, den) is
catastrophically cancellation-sensitive (operand rounding at even 2^-20
fails the 2e-2 budget), so those matmuls run as error-compensated fp32r
3-term products (hi/lo splits; hi*hi + hi*lo + lo*hi), which measures
bit-comparable to strict fp32 but streams at 1 cycle/row instead of 4.
The numerator path (v, M~, Pc=M~.pw, num) runs bf16.

Structure per (b, h): conv accumulates 9 fp32r taps/terms per 512-chunk in
PSUM; ACT evacuates hi (rounds to fp32r) + DVE computes lo. Sketches are
3-term fp32r. s_k is 16 one-column matmuls (rhs=ones), den is 16 one-column
matmuls (rhs=s_k) in [l,1] layout, so the divide becomes a per-partition
tensor_scalar and no L-wide broadcast matmul is needed. M~^T is accumulated
directly in [d, r] layout (lhsT=v tiles) so Pc = (M~^T)^T pw needs no
transpose. Batches are staggered (b1 conv emitted before b0 finale) to hide
the finale's elementwise latency under conv matmuls.
"""

import numpy as np
from contextlib import ExitStack

import concourse.bacc as bacc
import concourse.mybir as mybir
import concourse.tile as tile
from concourse.bass_utils import run_bass_kernel_spmd

F32 = mybir.dt.float32
F32R = mybir.dt.float32r
BF16 = mybir.dt.bfloat16
AF = mybir.ActivationFunctionType
ALU = mybir.AluOpType

B = 2
D = 128
L = 2048
H = 8
R = 128
KS = 3
LP = L + KS - 1
NCH = L // 512
NT = L // 128
SQRT_R = float(np.sqrt(R))

# f32r blob layout (free-dim offsets)
OFF_QKWH = 0                       # [2, 3, 128]
OFF_QKWL = OFF_QKWH + 2 * KS * D   # 768
OFF_GQH = OFF_QKWL + 2 * KS * D    # 1536: [2, 128]
OFF_GQL = OFF_GQH + 2 * R          # 1792
OFF_GKH = OFF_GQL + 2 * R          # 2048: [256] = [g1k | g2k]
OFF_GKL = OFF_GKH + 2 * R          # 2304
OFF_QKB = OFF_GKL + 2 * R          # 2560: [2]
OFF_X0H = OFF_QKB + 2              # 2562
OFF_X0L = OFF_X0H + LP             # 4612
BLOB_W = OFF_X0L + LP              # 6662
# x1 blob: [x1h | x1l]
XB_W = 2 * LP
# bf16 blob: [vw | pw | xb0 | xb1]
BOFF_VW = 0
BOFF_PW = BOFF_VW + D
BOFF_X0 = BOFF_PW + D
BBLOB_W = BOFF_X0 + 2 * LP

_built_nc = None
last_results = None


def _build():
    nc = bacc.Bacc(None, target_bir_lowering=False)
    blob_d = nc.declare_dram_parameter("blob", [D, BLOB_W], F32R, isOutput=False)
    xb_d = nc.declare_dram_parameter("xblob", [D, XB_W], F32R, isOutput=False)
    bblob_d = nc.declare_dram_parameter("bblob", [D, BBLOB_W], BF16, isOutput=False)
    out_d = nc.declare_dram_parameter("outp", [B, 128, NT, D], F32, isOutput=True)

    with ExitStack() as ctx:
        tc = ctx.enter_context(tile.TileContext(nc))
        consts = ctx.enter_context(tc.tile_pool(name="consts", bufs=1))
        data = ctx.enter_context(tc.tile_pool(name="data", bufs=1))
        work = ctx.enter_context(tc.tile_pool(name="work", bufs=3))
        psA = ctx.enter_context(tc.tile_pool(name="psA", bufs=3, space="PSUM"))
        psK = ctx.enter_context(tc.tile_pool(name="psK", bufs=2, space="PSUM"))
        psV = ctx.enter_context(tc.tile_pool(name="psV", bufs=1, space="PSUM"))
        psM = ctx.enter_context(tc.tile_pool(name="psM", bufs=1, space="PSUM"))
        psN = ctx.enter_context(tc.tile_pool(name="psN", bufs=1, space="PSUM"))

        wt = consts.tile([D, BLOB_W], F32R, tag="wt")
        nc.gpsimd.dma_start(out=wt[:, 0:OFF_X0H], in_=blob_d[:, 0:OFF_X0H])
        for s, e in ((OFF_X0H, OFF_X0H + 1025), (OFF_X0H + 1025, OFF_X0L),
                     (OFF_X0L, OFF_X0L + 1025), (OFF_X0L + 1025, BLOB_W)):
            nc.sync.dma_start(out=wt[:, s:e], in_=blob_d[:, s:e])
        x1t = consts.tile([D, XB_W], F32R, tag="x1t")
        nc.scalar.dma_start(out=x1t[:, 0:LP], in_=xb_d[:, 0:LP])
        nc.scalar.dma_start(out=x1t[:, LP:XB_W], in_=xb_d[:, LP:XB_W])
        bb = consts.tile([D, BBLOB_W], BF16, tag="bb")
        nc.gpsimd.dma_start(out=bb, in_=bblob_d[:])
        ones = consts.tile([D, 1], F32, tag="ones")
        nc.vector.memset(ones, 1.0)

        qkwh = wt[:, OFF_QKWH:OFF_QKWL].rearrange("p (a t d) -> p a t d", a=2, t=KS)
        qkwl = wt[:, OFF_QKWL:OFF_GQH].rearrange("p (a t d) -> p a t d", a=2, t=KS)
        gqh = wt[:, OFF_GQH:OFF_GQL].rearrange("p (a r) -> p a r", a=2)
        gql = wt[:, OFF_GQL:OFF_GKH].rearrange("p (a r) -> p a r", a=2)
        gkh = wt[:, OFF_GKH:OFF_GKL]
        gkl = wt[:, OFF_GKL:OFF_QKB]
        qkb = wt[:, OFF_QKB:OFF_QKB + 2]
        xh = [wt[:, OFF_X0H:OFF_X0L], x1t[:, 0:LP]]
        xl = [wt[:, OFF_X0L:BLOB_W], x1t[:, LP:XB_W]]
        vw_b = bb[:, BOFF_VW:BOFF_VW + D]
        pw_b = bb[:, BOFF_PW:BOFF_PW + D]
        xb = [bb[:, BOFF_X0 + b * LP:BOFF_X0 + (b + 1) * LP] for b in range(B)]

        # per-batch tiles
        t_qkh, t_qkl, t_phiq, t_phiqb, t_phik, t_phikb, t_vau, t_ost = (
            [], [], [], [], [], [], [], [])
        t_mt, t_pc, t_sk, t_rd = [], [], [], []
        for b in range(B):
            t_qkh.append(data.tile([D, 2, L], F32R, tag=f"qkh{b}", name=f"qkh{b}"))
            t_qkl.append(data.tile([D, 2, L], F32R, tag=f"qkl{b}", name=f"qkl{b}"))
            t_phiq.append(data.tile([R, L], F32, tag=f"phiq{b}", name=f"phiq{b}"))
            t_phiqb.append(data.tile([R, L], BF16, tag=f"phiqb{b}", name=f"phiqb{b}"))
            t_phik.append(data.tile([128, NT, R], F32, tag=f"phik{b}", name=f"phik{b}"))
            t_phikb.append(data.tile([128, NT, R], BF16, tag=f"phikb{b}", name=f"phikb{b}"))
            t_vau.append(data.tile([128, NT, D], BF16, tag=f"vau{b}", name=f"vau{b}"))
            t_ost.append(data.tile([128, NT, D], F32, tag=f"ost{b}", name=f"ost{b}"))
            t_mt.append(data.tile([D, R], BF16, tag=f"mt{b}", name=f"mt{b}"))
            t_pc.append(data.tile([R, D], BF16, tag=f"pc{b}", name=f"pc{b}"))
            t_sk.append(data.tile([R, 1], F32, tag=f"sk{b}", name=f"sk{b}"))
            t_rd.append(data.tile([128, NT], F32, tag=f"rd{b}", name=f"rd{b}"))

        def conv(b):
            qk_h, qk_l = t_qkh[b], t_qkl[b]
            for p in range(2):
                for c in range(NCH):
                    ps = psA.tile([128, 512], F32, tag="psA")
                    first = True
                    for t in range(KS):
                        rh = xh[b][:, c * 512 + t:c * 512 + t + 512]
                        rl = xl[b][:, c * 512 + t:c * 512 + t + 512]
                        nc.tensor.matmul(ps, lhsT=qkwh[:, p, t, :], rhs=rh,
                                         start=first, stop=False)
                        first = False
                        nc.tensor.matmul(ps, lhsT=qkwh[:, p, t, :], rhs=rl,
                                         start=False, stop=False)
                        nc.tensor.matmul(ps, lhsT=qkwl[:, p, t, :], rhs=rh,
                                         start=False, stop=(t == KS - 1))
                    hs = qk_h[:, p, c * 512:(c + 1) * 512]
                    nc.scalar.add(hs, ps, qkb[:, p:p + 1])
                    nc.vector.scalar_tensor_tensor(
                        qk_l[:, p, c * 512:(c + 1) * 512], ps, qkb[:, p:p + 1],
                        hs, op0=ALU.add, op1=ALU.subtract)

        def sketch_q(b):
            qk_h, qk_l, phiq = t_qkh[b], t_qkl[b], t_phiq[b]
            for c in range(NCH):
                rh = qk_h[:, 0, c * 512:(c + 1) * 512]
                rl = qk_l[:, 0, c * 512:(c + 1) * 512]
                us = []
                for g in range(2):
                    u = psA.tile([128, 512], F32, tag="psA")
                    nc.tensor.matmul(u, lhsT=gqh[:, g, :], rhs=rh,
                                     start=True, stop=False)
                    nc.tensor.matmul(u, lhsT=gqh[:, g, :], rhs=rl,
                                     start=False, stop=False)
                    nc.tensor.matmul(u, lhsT=gql[:, g, :], rhs=rh,
                                     start=False, stop=True)
                    us.append(u)
                u1s = work.tile([128, 512], F32, tag="u1s")
                nc.scalar.copy(u1s, us[0])
                nc.vector.tensor_mul(phiq[:, c * 512:(c + 1) * 512], u1s, us[1])
            phiqb = t_phiqb[b]
            for hh in range(2):
                sl = slice(hh * (L // 2), (hh + 1) * (L // 2))
                nc.scalar.activation(phiqb[:, sl], phiq[:, sl], AF.Tanh,
                                     scale=1.0 / SQRT_R)
                nc.scalar.activation(phiq[:, sl], phiq[:, sl], AF.Tanh,
                                     scale=1.0 / SQRT_R)

        def sketch_k(b):
            qk_h, qk_l, phik = t_qkh[b], t_qkl[b], t_phik[b]
            for mg in range(NT // 2):
                uu = psK.tile([128, 2, 256], F32, tag="uu")
                for j in range(2):
                    m = mg * 2 + j
                    klh = qk_h[:, 1, m * 128:(m + 1) * 128]
                    kll = qk_l[:, 1, m * 128:(m + 1) * 128]
                    nc.tensor.matmul(uu[:, j, :], lhsT=klh, rhs=gkh,
                                     start=(j == 0), stop=False,
                                     skip_group_check=True)
                    nc.tensor.matmul(uu[:, j, :], lhsT=kll, rhs=gkh,
                                     start=False, stop=False,
                                     skip_group_check=True)
                    nc.tensor.matmul(uu[:, j, :], lhsT=klh, rhs=gkl,
                                     start=False, stop=(j == 1),
                                     skip_group_check=True)
                u1k = work.tile([128, 2, 128], F32, tag="u1k")
                nc.scalar.copy(u1k, uu[:, :, 0:128])
                nc.vector.tensor_mul(phik[:, mg * 2:(mg + 1) * 2, :], u1k,
                                     uu[:, :, 128:256])
            pf = phik.rearrange("p a b -> p (a b)")
            for hh in range(2):
                sl = slice(hh * (NT // 2) * R, (hh + 1) * (NT // 2) * R)
                nc.scalar.activation(pf[:, sl], pf[:, sl], AF.Tanh,
                                     scale=1.0 / SQRT_R)
            nc.gpsimd.tensor_copy(
                t_phikb[b].rearrange("p a b -> p (a b)"), pf)

        def vconv(b):
            vau = t_vau[b]
            for vg in range(NT // 4):
                vp = psV.tile([128, 4, D], F32, tag="vp")
                for j in range(4):
                    m = vg * 4 + j
                    nc.tensor.matmul(
                        vp[:, j, :],
                        lhsT=xb[b][:, KS - 1 + m * 128:KS - 1 + (m + 1) * 128],
                        rhs=vw_b, start=(j == 0), stop=(j == 3),
                        skip_group_check=True)
                nc.scalar.copy(vau[:, vg * 4:(vg + 1) * 4, :], vp)

        def finale(b):
            phiq, phiqb, phik, phikb = (t_phiq[b], t_phiqb[b], t_phik[b],
                                        t_phikb[b])
            vau, ost = t_vau[b], t_ost[b]
            mt_sb, pc_sb, sk_sb, rd = t_mt[b], t_pc[b], t_sk[b], t_rd[b]
            # M~^T [d, r] + s_k [r, 1] share one psum bank/group
            mtile = psM.tile([128, 512], F32, tag="psM")
            for m in range(NT):
                nc.tensor.matmul(mtile[:, 0:R], lhsT=vau[:, m, :],
                                 rhs=phikb[:, m, :], start=(m == 0),
                                 stop=False, skip_group_check=True)
            for m in range(NT):
                nc.tensor.matmul(mtile[:, R:R + 1], lhsT=phik[:, m, :],
                                 rhs=ones, start=False, stop=(m == NT - 1),
                                 skip_group_check=True)
            nc.scalar.copy(mt_sb, mtile[:, 0:R])
            nc.vector.tensor_copy(sk_sb, mtile[:, R:R + 1])
            # Pc [r, j] = M~ @ pw
            pcp = psM.tile([128, 512], F32, tag="psM")
            nc.tensor.matmul(pcp[:, 0:D], lhsT=mt_sb, rhs=pw_b,
                             start=True, stop=True, skip_group_check=True)
            nc.scalar.copy(pc_sb, pcp[:, 0:D])
            # den columns [m-tile, 1]
            dn = psM.tile([128, 512], F32, tag="psM")
            for m in range(NT):
                nc.tensor.matmul(dn[:, m:m + 1],
                                 lhsT=phiq[:, m * 128:(m + 1) * 128],
                                 rhs=sk_sb, start=(m == 0),
                                 stop=(m == NT - 1), skip_group_check=True)
            nc.vector.reciprocal(rd, dn[:, 0:NT])
            # num [m, j] groups of 4 + divide + ship
            for mg in range(NT // 4):
                nps = psN.tile([128, 4, D], F32, tag="psN")
                for j in range(4):
                    m = mg * 4 + j
                    nc.tensor.matmul(nps[:, j, :],
                                     lhsT=phiqb[:, m * 128:(m + 1) * 128],
                                     rhs=pc_sb, start=(j == 0),
                                     stop=(j == 3), skip_group_check=True)
                for j in range(4):
                    m = mg * 4 + j
                    nc.vector.tensor_scalar(ost[:, m, :], nps[:, j, :],
                                            rd[:, m:m + 1], None, op0=ALU.mult)
                eng = nc.scalar if mg % 2 == 0 else nc.sync
                eng.dma_start(out=out_d[b, :, mg * 4:(mg + 1) * 4, :],
                              in_=ost[:, mg * 4:(mg + 1) * 4, :])

        # staggered emission: hide b0 finale latency under b1 conv
        conv(0)
        sketch_q(0)
        sketch_k(0)
        vconv(0)
        conv(1)
        finale(0)
        sketch_q(1)
        sketch_k(1)
        vconv(1)
        finale(1)
    nc.compile()
    return nc


def _split12(a):
    """Round-half-up split at 12 mantissa bits: a = hi + lo exactly, with hi
    representable in the PE's fp32r operand precision (probe-verified)."""
    a = np.ascontiguousarray(np.asarray(a, np.float32))
    u = a.view(np.uint32)
    hi = ((u + np.uint32(0x800)) & np.uint32(0xFFFFF000)).view(np.float32).copy()
    lo = (a - hi).astype(np.float32)
    return hi, lo


def _prep_in_maps(inputs):
    import ml_dtypes

    def f32(a):
        return np.ascontiguousarray(np.asarray(a), dtype=np.float32)

    x = f32(inputs["x"])
    q_w = f32(inputs["q_w"]).reshape(H, D, D, KS)
    k_w = f32(inputs["k_w"]).reshape(H, D, D, KS)
    v_w = f32(inputs["v_w"]).reshape(H, D, D)
    q_b = f32(inputs["q_b"]).reshape(H, D)
    k_b = f32(inputs["k_b"]).reshape(H, D)
    proj_w = f32(inputs["proj_w"])
    gq = float(np.asarray(inputs["gamma_q"]).reshape(-1)[0])
    bq = float(np.asarray(inputs["beta_q"]).reshape(-1)[0])
    gk = float(np.asarray(inputs["gamma_k"]).reshape(-1)[0])
    bk = float(np.asarray(inputs["beta_k"]).reshape(-1)[0])

    xp = np.zeros((D, B, LP), np.float32)
    xp[:, :, KS - 1:] = x.transpose(1, 0, 2)
    xp_h, xp_l = _split12(xp)
    gq_s = np.stack([f32(inputs["g1_q"]), f32(inputs["g2_q"])], axis=1)  # [D,2,R]
    gk_s = np.concatenate([f32(inputs["g1_k"]), f32(inputs["g2_k"])], axis=1)
    gq_h, gq_l = _split12(gq_s)
    gk_h, gk_l = _split12(gk_s)

    in_maps = []
    for h in range(H):
        blob = np.zeros((D, BLOB_W), np.float32)
        wq_h, wq_l = _split12((gq * q_w[h]).transpose(1, 2, 0))  # [c, t, d]
        wk_h, wk_l = _split12((gk * k_w[h]).transpose(1, 2, 0))
        qh = blob[:, OFF_QKWH:OFF_QKWL].reshape(D, 2, KS, D)
        qh[:, 0], qh[:, 1] = wq_h, wk_h
        ql = blob[:, OFF_QKWL:OFF_GQH].reshape(D, 2, KS, D)
        ql[:, 0], ql[:, 1] = wq_l, wk_l
        blob[:, OFF_GQH:OFF_GQL] = gq_h.reshape(D, 2 * R)
        blob[:, OFF_GQL:OFF_GKH] = gq_l.reshape(D, 2 * R)
        blob[:, OFF_GKH:OFF_GKL] = gk_h
        blob[:, OFF_GKL:OFF_QKB] = gk_l
        blob[:, OFF_QKB] = gq * q_b[h] + bq
        blob[:, OFF_QKB + 1] = gk * k_b[h] + bk
        blob[:, OFF_X0H:OFF_X0L] = xp_h[:, 0]
        blob[:, OFF_X0L:BLOB_W] = xp_l[:, 0]
        xblob = np.empty((D, XB_W), np.float32)
        xblob[:, 0:LP] = xp_h[:, 1]
        xblob[:, LP:XB_W] = xp_l[:, 1]
        bblob = np.empty((D, BBLOB_W), ml_dtypes.bfloat16)
        bblob[:, BOFF_VW:BOFF_VW + D] = v_w[h].T.astype(ml_dtypes.bfloat16)
        bblob[:, BOFF_PW:BOFF_PW + D] = (
            proj_w[:, h * D:(h + 1) * D].T.astype(ml_dtypes.bfloat16))
        bblob[:, BOFF_X0:BOFF_X0 + LP] = xp[:, 0].astype(ml_dtypes.bfloat16)
        bblob[:, BOFF_X0 + LP:] = xp[:, 1].astype(ml_dtypes.bfloat16)
        in_maps.append(dict(blob=blob, xblob=xblob, bblob=bblob))
    return in_maps


def kernel(**inputs):
    global _built_nc, last_results
    if _built_nc is None:
        _built_nc = _build()
    in_maps = _prep_in_maps(inputs)
    res = run_bass_kernel_spmd(_built_nc, in_maps, list(range(H)))
    last_results = res
    parts = np.stack([res.results[c]["outp"] for c in range(H)])  # [H,B,128,NT,D]
    out = parts.sum(axis=0, dtype=np.float32)        # [B, 128(p), NT(m), D]
    out = out.transpose(0, 2, 1, 3).reshape(B, L, D)  # l = m*128 + p
    out = np.ascontiguousarray(out)
    out += np.asarray(inputs["proj_b"], np.float32)[None, None, :]
    return out.astype(np.float32)


# revision 11
# speedup vs baseline: 1.1633x; 1.1633x over previous
"""Conv1D-MHSA (sketched linear attention) Trainium2 kernel, v2.

Math (per batch b, head h; head h -> core h):
    q = conv1d_K3(x_pad, q_w) ; k likewise ; v = conv1d_K1(x, v_w)
    phi_q = tanh((q^T g1_q)*(q^T g2_q)/sqrt(R))  (phi_k likewise; the sqrt(R)
    post-scales cancel between numerator and denominator, eps -> ~0 dropped)
    out_h = diag(1/(phi_q^T s_k)) . phi_q^T (M~ pw),  M~ = phi_k^T v,
    s_k = colsum(phi_k);  host sums the 8 per-head partials + proj_b.

Precision: the denominator path (conv, sketches, phi_q, s_k, den) is
catastrophically cancellation-sensitive (operand rounding at even 2^-20
fails the 2e-2 budget), so those matmuls run as error-compensated fp32r
3-term products (hi/lo splits; hi*hi + hi*lo + lo*hi), which measures
bit-comparable to strict fp32 but streams at 1 cycle/row instead of 4.
The numerator path (v, M~, Pc=M~.pw, num) runs bf16.

Structure per (b, h): conv accumulates 9 fp32r taps/terms per 512-chunk in
PSUM; ACT evacuates hi (rounds to fp32r) + DVE computes lo. Sketches are
3-term fp32r. s_k is 16 one-column matmuls (rhs=ones), den is 16 one-column
matmuls (rhs=s_k) in [l,1] layout, so the divide becomes a per-partition
tensor_scalar and no L-wide broadcast matmul is needed. M~^T is accumulated
directly in [d, r] layout (lhsT=v tiles) so Pc = (M~^T)^T pw needs no
transpose. Batches are staggered (b1 conv emitted before b0 finale) to hide
the finale's elementwise latency under conv matmuls.
"""

import numpy as np
from contextlib import ExitStack

import concourse.bacc as bacc
import concourse.mybir as mybir
import concourse.tile as tile
from concourse.bass_utils import run_bass_kernel_spmd

F32 = mybir.dt.float32
F32R = mybir.dt.float32r
BF16 = mybir.dt.bfloat16
AF = mybir.ActivationFunctionType
ALU = mybir.AluOpType

B = 2
D = 128
L = 2048
H = 8
R = 128
KS = 3
LP = L + KS - 1
NCH = L // 512
NT = L // 128
SQRT_R = float(np.sqrt(R))

# f32r blob layout (free-dim offsets)
OFF_QKWH = 0                       # [2, 3, 128]
OFF_QKWL = OFF_QKWH + 2 * KS * D   # 768
OFF_GQH = OFF_QKWL + 2 * KS * D    # 1536: [2, 128]
OFF_GQL = OFF_GQH + 2 * R          # 1792
OFF_GKH = OFF_GQL + 2 * R          # 2048: [256] = [g1k | g2k]
OFF_GKL = OFF_GKH + 2 * R          # 2304
OFF_QKB = OFF_GKL + 2 * R          # 2560: [2]
OFF_X0H = OFF_QKB + 2              # 2562
OFF_X0L = OFF_X0H + LP             # 4612
BLOB_W = OFF_X0L + LP              # 6662
# x1 blob: [x1h | x1l]
XB_W = 2 * LP
# bf16 blob: [vw | pw | xb0 | xb1]
BOFF_VW = 0
BOFF_PW = BOFF_VW + D
BOFF_X0 = BOFF_PW + D
BBLOB_W = BOFF_X0 + 2 * LP

_built_nc = None
last_results = None


def _build():
    nc = bacc.Bacc(None, target_bir_lowering=False)
    blob_d = nc.declare_dram_parameter("blob", [D, BLOB_W], F32R, isOutput=False)
    xb_d = nc.declare_dram_parameter("xblob", [D, XB_W], F32R, isOutput=False)
    bblob_d = nc.declare_dram_parameter("bblob", [D, BBLOB_W], BF16, isOutput=False)
    out_d = nc.declare_dram_parameter("outp", [B, 128, NT, D], F32, isOutput=True)

    with ExitStack() as ctx:
        tc = ctx.enter_context(tile.TileContext(nc))
        consts = ctx.enter_context(tc.tile_pool(name="consts", bufs=1))
        data = ctx.enter_context(tc.tile_pool(name="data", bufs=1))
        work = ctx.enter_context(tc.tile_pool(name="work", bufs=3))
        psA = ctx.enter_context(tc.tile_pool(name="psA", bufs=2, space="PSUM"))
        psK = ctx.enter_context(tc.tile_pool(name="psK", bufs=2, space="PSUM"))
        psV = ctx.enter_context(tc.tile_pool(name="psV", bufs=1, space="PSUM"))
        psM = ctx.enter_context(tc.tile_pool(name="psM", bufs=1, space="PSUM"))
        psN = ctx.enter_context(tc.tile_pool(name="psN", bufs=2, space="PSUM"))

        wt = consts.tile([D, BLOB_W], F32R, tag="wt")
        # weights split so the first conv terms unblock ASAP
        nc.gpsimd.dma_start(out=wt[:, 0:OFF_QKWL], in_=blob_d[:, 0:OFF_QKWL])
        nc.gpsimd.dma_start(out=wt[:, OFF_QKWL:OFF_GQH],
                            in_=blob_d[:, OFF_QKWL:OFF_GQH])
        nc.gpsimd.dma_start(out=wt[:, OFF_GQH:OFF_X0H],
                            in_=blob_d[:, OFF_GQH:OFF_X0H])
        # x0 hi/lo interleaved quarters: conv chunk c needs hi+lo of its span
        qs = (0, 515, 1027, 1539, LP)
        for i in range(4):
            nc.sync.dma_start(out=wt[:, OFF_X0H + qs[i]:OFF_X0H + qs[i + 1]],
                              in_=blob_d[:, OFF_X0H + qs[i]:OFF_X0H + qs[i + 1]])
            nc.sync.dma_start(out=wt[:, OFF_X0L + qs[i]:OFF_X0L + qs[i + 1]],
                              in_=blob_d[:, OFF_X0L + qs[i]:OFF_X0L + qs[i + 1]])
        x1t = consts.tile([D, XB_W], F32R, tag="x1t")
        nc.scalar.dma_start(out=x1t[:, 0:LP], in_=xb_d[:, 0:LP])
        nc.scalar.dma_start(out=x1t[:, LP:XB_W], in_=xb_d[:, LP:XB_W])
        bb = consts.tile([D, BBLOB_W], BF16, tag="bb")
        nc.gpsimd.dma_start(out=bb, in_=bblob_d[:])
        ones = consts.tile([D, 1], F32, tag="ones")
        nc.vector.memset(ones, 1.0)

        qkwh = wt[:, OFF_QKWH:OFF_QKWL].rearrange("p (a t d) -> p a t d", a=2, t=KS)
        qkwl = wt[:, OFF_QKWL:OFF_GQH].rearrange("p (a t d) -> p a t d", a=2, t=KS)
        gqh = wt[:, OFF_GQH:OFF_GQL].rearrange("p (a r) -> p a r", a=2)
        gql = wt[:, OFF_GQL:OFF_GKH].rearrange("p (a r) -> p a r", a=2)
        gkh = wt[:, OFF_GKH:OFF_GKL]
        gkl = wt[:, OFF_GKL:OFF_QKB]
        qkb = wt[:, OFF_QKB:OFF_QKB + 2]
        xh = [wt[:, OFF_X0H:OFF_X0L], x1t[:, 0:LP]]
        xl = [wt[:, OFF_X0L:BLOB_W], x1t[:, LP:XB_W]]
        vw_b = bb[:, BOFF_VW:BOFF_VW + D]
        pw_b = bb[:, BOFF_PW:BOFF_PW + D]
        xb = [bb[:, BOFF_X0 + b * LP:BOFF_X0 + (b + 1) * LP] for b in range(B)]

        # per-batch tiles
        t_qkh, t_qkl, t_phiq, t_phiqb, t_phik, t_phikb, t_vau, t_ost = (
            [], [], [], [], [], [], [], [])
        t_mt, t_pc, t_sk, t_rd = [], [], [], []
        for b in range(B):
            t_qkh.append(data.tile([D, 2, L], F32R, tag=f"qkh{b}", name=f"qkh{b}"))
            t_qkl.append(data.tile([D, 2, L], F32R, tag=f"qkl{b}", name=f"qkl{b}"))
            t_phiq.append(data.tile([R, L], F32, tag=f"phiq{b}", name=f"phiq{b}"))
            t_phiqb.append(data.tile([R, L], BF16, tag=f"phiqb{b}", name=f"phiqb{b}"))
            t_phik.append(data.tile([128, NT, R], F32, tag=f"phik{b}", name=f"phik{b}"))
            t_phikb.append(data.tile([128, NT, R], BF16, tag=f"phikb{b}", name=f"phikb{b}"))
            t_vau.append(data.tile([128, NT, D], BF16, tag=f"vau{b}", name=f"vau{b}"))
            t_ost.append(data.tile([128, NT, D], F32, tag=f"ost{b}", name=f"ost{b}"))
            t_mt.append(data.tile([D, R], BF16, tag=f"mt{b}", name=f"mt{b}"))
            t_pc.append(data.tile([R, D], BF16, tag=f"pc{b}", name=f"pc{b}"))
            t_sk.append(data.tile([R, 1], F32, tag=f"sk{b}", name=f"sk{b}"))
            t_rd.append(data.tile([128, NT], F32, tag=f"rd{b}", name=f"rd{b}"))

        def conv(b):
            qk_h, qk_l = t_qkh[b], t_qkl[b]
            for p in range(2):
                for c in range(NCH):
                    ps = psA.tile([128, 512], F32, tag="psA")
                    first = True
                    for t in range(KS):
                        rh = xh[b][:, c * 512 + t:c * 512 + t + 512]
                        rl = xl[b][:, c * 512 + t:c * 512 + t + 512]
                        nc.tensor.matmul(ps, lhsT=qkwh[:, p, t, :], rhs=rh,
                                         start=first, stop=False)
                        first = False
                        nc.tensor.matmul(ps, lhsT=qkwh[:, p, t, :], rhs=rl,
                                         start=False, stop=False)
                        nc.tensor.matmul(ps, lhsT=qkwl[:, p, t, :], rhs=rh,
                                         start=False, stop=(t == KS - 1))
                    hs = qk_h[:, p, c * 512:(c + 1) * 512]
                    nc.scalar.add(hs, ps, qkb[:, p:p + 1])
                    nc.vector.scalar_tensor_tensor(
                        qk_l[:, p, c * 512:(c + 1) * 512], ps, qkb[:, p:p + 1],
                        hs, op0=ALU.add, op1=ALU.subtract)

        def sketch_q(b):
            qk_h, qk_l, phiq = t_qkh[b], t_qkl[b], t_phiq[b]
            for c in range(NCH):
                rh = qk_h[:, 0, c * 512:(c + 1) * 512]
                rl = qk_l[:, 0, c * 512:(c + 1) * 512]
                us = []
                for g in range(2):
                    u = psA.tile([128, 512], F32, tag="psA")
                    nc.tensor.matmul(u, lhsT=gqh[:, g, :], rhs=rh,
                                     start=True, stop=False)
                    nc.tensor.matmul(u, lhsT=gqh[:, g, :], rhs=rl,
                                     start=False, stop=False)
                    nc.tensor.matmul(u, lhsT=gql[:, g, :], rhs=rh,
                                     start=False, stop=True)
                    us.append(u)
                u1s = work.tile([128, 512], F32, tag="u1s")
                nc.scalar.copy(u1s, us[0])
                nc.vector.tensor_mul(phiq[:, c * 512:(c + 1) * 512], u1s, us[1])
            phiqb = t_phiqb[b]
            for hh in range(2):
                sl = slice(hh * (L // 2), (hh + 1) * (L // 2))
                nc.scalar.activation(phiq[:, sl], phiq[:, sl], AF.Tanh,
                                     scale=1.0 / SQRT_R)
                nc.gpsimd.tensor_copy(phiqb[:, sl], phiq[:, sl])

        def sketch_k(b):
            qk_h, qk_l, phik = t_qkh[b], t_qkl[b], t_phik[b]
            for mg in range(NT // 2):
                uu = psK.tile([128, 2, 256], F32, tag="uu")
                for j in range(2):
                    m = mg * 2 + j
                    klh = qk_h[:, 1, m * 128:(m + 1) * 128]
                    kll = qk_l[:, 1, m * 128:(m + 1) * 128]
                    nc.tensor.matmul(uu[:, j, :], lhsT=klh, rhs=gkh,
                                     start=(j == 0), stop=False,
                                     skip_group_check=True)
                    nc.tensor.matmul(uu[:, j, :], lhsT=kll, rhs=gkh,
                                     start=False, stop=False,
                                     skip_group_check=True)
                    nc.tensor.matmul(uu[:, j, :], lhsT=klh, rhs=gkl,
                                     start=False, stop=(j == 1),
                                     skip_group_check=True)
                u1k = work.tile([128, 2, 128], F32, tag="u1k")
                nc.scalar.copy(u1k, uu[:, :, 0:128])
                nc.vector.tensor_mul(phik[:, mg * 2:(mg + 1) * 2, :], u1k,
                                     uu[:, :, 128:256])
            pf = phik.rearrange("p a b -> p (a b)")
            pfb = t_phikb[b].rearrange("p a b -> p (a b)")
            for hh in range(2):
                sl = slice(hh * (NT // 2) * R, (hh + 1) * (NT // 2) * R)
                nc.scalar.activation(pf[:, sl], pf[:, sl], AF.Tanh,
                                     scale=1.0 / SQRT_R)
                nc.gpsimd.tensor_copy(pfb[:, sl], pf[:, sl])

        def vconv(b):
            vau = t_vau[b]
            for vg in range(NT // 4):
                vp = psV.tile([128, 4, D], F32, tag="vp")
                for j in range(4):
                    m = vg * 4 + j
                    nc.tensor.matmul(
                        vp[:, j, :],
                        lhsT=xb[b][:, KS - 1 + m * 128:KS - 1 + (m + 1) * 128],
                        rhs=vw_b, start=(j == 0), stop=(j == 3),
                        skip_group_check=True)
                nc.scalar.copy(vau[:, vg * 4:(vg + 1) * 4, :], vp)

        def finale(b):
            phiq, phiqb, phik, phikb = (t_phiq[b], t_phiqb[b], t_phik[b],
                                        t_phikb[b])
            vau, ost = t_vau[b], t_ost[b]
            mt_sb, pc_sb, sk_sb, rd = t_mt[b], t_pc[b], t_sk[b], t_rd[b]
            # M~^T [d, r] + s_k [r, 1] share one psum bank/group
            mtile = psM.tile([128, 512], F32, tag="psM")
            for m in range(NT):
                nc.tensor.matmul(mtile[:, 0:R], lhsT=vau[:, m, :],
                                 rhs=phikb[:, m, :], start=(m == 0),
                                 stop=False, skip_group_check=True)
            for m in range(NT):
                nc.tensor.matmul(mtile[:, R:R + 1], lhsT=phik[:, m, :],
                                 rhs=ones, start=False, stop=(m == NT - 1),
                                 skip_group_check=True)
            nc.scalar.copy(mt_sb, mtile[:, 0:R])
            nc.vector.tensor_copy(sk_sb, mtile[:, R:R + 1])
            # Pc [r, j] = M~ @ pw
            pcp = psM.tile([128, 512], F32, tag="psM")
            nc.tensor.matmul(pcp[:, 0:D], lhsT=mt_sb, rhs=pw_b,
                             start=True, stop=True, skip_group_check=True)
            nc.scalar.copy(pc_sb, pcp[:, 0:D])
            # den columns [m-tile, 1]
            dn = psM.tile([128, 512], F32, tag="psM")
            for m in range(NT):
                nc.tensor.matmul(dn[:, m:m + 1],
                                 lhsT=phiq[:, m * 128:(m + 1) * 128],
                                 rhs=sk_sb, start=(m == 0),
                                 stop=(m == NT - 1), skip_group_check=True)
            nc.vector.reciprocal(rd, dn[:, 0:NT])
            # num [m, j] groups of 4 + divide + ship
            for mg in range(NT // 4):
                nps = psN.tile([128, 4, D], F32, tag="psN")
                for j in range(4):
                    m = mg * 4 + j
                    nc.tensor.matmul(nps[:, j, :],
                                     lhsT=phiqb[:, m * 128:(m + 1) * 128],
                                     rhs=pc_sb, start=(j == 0),
                                     stop=(j == 3), skip_group_check=True)
                for j in range(4):
                    m = mg * 4 + j
                    if j % 2 == 0:
                        nc.vector.tensor_scalar(ost[:, m, :], nps[:, j, :],
                                                rd[:, m:m + 1], None,
                                                op0=ALU.mult)
                    else:
                        nc.scalar.activation(ost[:, m, :], nps[:, j, :],
                                             AF.Identity,
                                             scale=rd[:, m:m + 1])
                eng = nc.scalar if mg % 2 == 0 else nc.sync
                eng.dma_start(out=out_d[b, :, mg * 4:(mg + 1) * 4, :],
                              in_=ost[:, mg * 4:(mg + 1) * 4, :])

        # staggered emission: hide b0 finale latency under b1 conv
        conv(0)
        sketch_q(0)
        sketch_k(0)
        vconv(0)
        conv(1)
        finale(0)
        sketch_q(1)
        sketch_k(1)
        vconv(1)
        finale(1)
    nc.compile()
    return nc


def _split12(a):
    """Round-half-up split at 12 mantissa bits: a = hi + lo exactly, with hi
    representable in the PE's fp32r operand precision (probe-verified)."""
    a = np.ascontiguousarray(np.asarray(a, np.float32))
    u = a.view(np.uint32)
    hi = ((u + np.uint32(0x800)) & np.uint32(0xFFFFF000)).view(np.float32).copy()
    lo = (a - hi).astype(np.float32)
    return hi, lo


def _prep_in_maps(inputs):
    import ml_dtypes

    def f32(a):
        return np.ascontiguousarray(np.asarray(a), dtype=np.float32)

    x = f32(inputs["x"])
    q_w = f32(inputs["q_w"]).reshape(H, D, D, KS)
    k_w = f32(inputs["k_w"]).reshape(H, D, D, KS)
    v_w = f32(inputs["v_w"]).reshape(H, D, D)
    q_b = f32(inputs["q_b"]).reshape(H, D)
    k_b = f32(inputs["k_b"]).reshape(H, D)
    proj_w = f32(inputs["proj_w"])
    gq = float(np.asarray(inputs["gamma_q"]).reshape(-1)[0])
    bq = float(np.asarray(inputs["beta_q"]).reshape(-1)[0])
    gk = float(np.asarray(inputs["gamma_k"]).reshape(-1)[0])
    bk = float(np.asarray(inputs["beta_k"]).reshape(-1)[0])

    xp = np.zeros((D, B, LP), np.float32)
    xp[:, :, KS - 1:] = x.transpose(1, 0, 2)
    xp_h, xp_l = _split12(xp)
    gq_s = np.stack([f32(inputs["g1_q"]), f32(inputs["g2_q"])], axis=1)  # [D,2,R]
    gk_s = np.concatenate([f32(inputs["g1_k"]), f32(inputs["g2_k"])], axis=1)
    gq_h, gq_l = _split12(gq_s)
    gk_h, gk_l = _split12(gk_s)

    in_maps = []
    for h in range(H):
        blob = np.zeros((D, BLOB_W), np.float32)
        wq_h, wq_l = _split12((gq * q_w[h]).transpose(1, 2, 0))  # [c, t, d]
        wk_h, wk_l = _split12((gk * k_w[h]).transpose(1, 2, 0))
        qh = blob[:, OFF_QKWH:OFF_QKWL].reshape(D, 2, KS, D)
        qh[:, 0], qh[:, 1] = wq_h, wk_h
        ql = blob[:, OFF_QKWL:OFF_GQH].reshape(D, 2, KS, D)
        ql[:, 0], ql[:, 1] = wq_l, wk_l
        blob[:, OFF_GQH:OFF_GQL] = gq_h.reshape(D, 2 * R)
        blob[:, OFF_GQL:OFF_GKH] = gq_l.reshape(D, 2 * R)
        blob[:, OFF_GKH:OFF_GKL] = gk_h
        blob[:, OFF_GKL:OFF_QKB] = gk_l
        blob[:, OFF_QKB] = gq * q_b[h] + bq
        blob[:, OFF_QKB + 1] = gk * k_b[h] + bk
        blob[:, OFF_X0H:OFF_X0L] = xp_h[:, 0]
        blob[:, OFF_X0L:BLOB_W] = xp_l[:, 0]
        xblob = np.empty((D, XB_W), np.float32)
        xblob[:, 0:LP] = xp_h[:, 1]
        xblob[:, LP:XB_W] = xp_l[:, 1]
        bblob = np.empty((D, BBLOB_W), ml_dtypes.bfloat16)
        bblob[:, BOFF_VW:BOFF_VW + D] = v_w[h].T.astype(ml_dtypes.bfloat16)
        bblob[:, BOFF_PW:BOFF_PW + D] = (
            proj_w[:, h * D:(h + 1) * D].T.astype(ml_dtypes.bfloat16))
        bblob[:, BOFF_X0:BOFF_X0 + LP] = xp[:, 0].astype(ml_dtypes.bfloat16)
        bblob[:, BOFF_X0 + LP:] = xp[:, 1].astype(ml_dtypes.bfloat16)
        in_maps.append(dict(blob=blob, xblob=xblob, bblob=bblob))
    return in_maps


def kernel(**inputs):
    global _built_nc, last_results
    if _built_nc is None:
        _built_nc = _build()
    in_maps = _prep_in_maps(inputs)
    res = run_bass_kernel_spmd(_built_nc, in_maps, list(range(H)))
    last_results = res
    parts = np.stack([res.results[c]["outp"] for c in range(H)])  # [H,B,128,NT,D]
    out = parts.sum(axis=0, dtype=np.float32)        # [B, 128(p), NT(m), D]
    out = out.transpose(0, 2, 1, 3).reshape(B, L, D)  # l = m*128 + p
    out = np.ascontiguousarray(out)
    out += np.asarray(inputs["proj_b"], np.float32)[None, None, :]
    return out.astype(np.float32)


# revision 13
# speedup vs baseline: 1.3006x; 1.1180x over previous
"""Conv1D-MHSA (sketched linear attention) Trainium2 kernel, v2.

Math (per batch b, head h; head h -> core h):
    q = conv1d_K3(x_pad, q_w) ; k likewise ; v = conv1d_K1(x, v_w)
    phi_q = tanh((q^T g1_q)*(q^T g2_q)/sqrt(R))  (phi_k likewise; the sqrt(R)
    post-scales cancel between numerator and denominator, eps -> ~0 dropped)
    out_h = diag(1/(phi_q^T s_k)) . phi_q^T (M~ pw),  M~ = phi_k^T v,
    s_k = colsum(phi_k);  host sums the 8 per-head partials + proj_b.

Precision: the denominator path (conv, sketches, phi_q, s_k, den) is
catastrophically cancellation-sensitive (operand rounding at even 2^-20
fails the 2e-2 budget), so those matmuls run as error-compensated fp32r
3-term products (hi/lo splits; hi*hi + hi*lo + lo*hi), which measures
bit-comparable to strict fp32 but streams at 1 cycle/row instead of 4.
The numerator path (v, M~, Pc=M~.pw, num) runs bf16.

Structure per (b, h): conv accumulates 9 fp32r taps/terms per 512-chunk in
PSUM; ACT evacuates hi (rounds to fp32r) + DVE computes lo. Sketches are
3-term fp32r. s_k is 16 one-column matmuls (rhs=ones), den is 16 one-column
matmuls (rhs=s_k) in [l,1] layout, so the divide becomes a per-partition
tensor_scalar and no L-wide broadcast matmul is needed. M~^T is accumulated
directly in [d, r] layout (lhsT=v tiles) so Pc = (M~^T)^T pw needs no
transpose. Batches are staggered (b1 conv emitted before b0 finale) to hide
the finale's elementwise latency under conv matmuls.
"""

import numpy as np
from contextlib import ExitStack

import concourse.bacc as bacc
import concourse.mybir as mybir
import concourse.tile as tile
from concourse.bass_utils import run_bass_kernel_spmd

F32 = mybir.dt.float32
F32R = mybir.dt.float32r
BF16 = mybir.dt.bfloat16
AF = mybir.ActivationFunctionType
ALU = mybir.AluOpType

B = 2
D = 128
L = 2048
H = 8
R = 128
KS = 3
LP = L + KS - 1
NCH = L // 512
NT = L // 128
SQRT_R = float(np.sqrt(R))

# f32r blob layout (free-dim offsets)
OFF_QKWH = 0                       # [2, 3, 128]
OFF_QKWL = OFF_QKWH + 2 * KS * D   # 768
OFF_GQH = OFF_QKWL + 2 * KS * D    # 1536: [2, 128]
OFF_GQL = OFF_GQH + 2 * R          # 1792
OFF_GKH = OFF_GQL + 2 * R          # 2048: [256] = [g1k | g2k]
OFF_GKL = OFF_GKH + 2 * R          # 2304
OFF_QKB = OFF_GKL + 2 * R          # 2560: [2]
BLOB_W = OFF_QKB + 2               # 2562 (weights only)
XR_W = B * LP                      # raw x, both batches
BOFF_VW = 0
BOFF_PW = BOFF_VW + D
BBLOB_W = BOFF_PW + D

_built_nc = None
last_results = None


def _build():
    nc = bacc.Bacc(None, target_bir_lowering=False)
    blob_d = nc.declare_dram_parameter("blob", [D, BLOB_W], F32R, isOutput=False)
    xr_d = nc.declare_dram_parameter("xraw", [D, XR_W], F32, isOutput=False)
    bblob_d = nc.declare_dram_parameter("bblob", [D, BBLOB_W], BF16, isOutput=False)
    out_d = nc.declare_dram_parameter("outp", [B, 128, NT, D], BF16, isOutput=True)

    with ExitStack() as ctx:
        tc = ctx.enter_context(tile.TileContext(nc))
        consts = ctx.enter_context(tc.tile_pool(name="consts", bufs=1))
        data = ctx.enter_context(tc.tile_pool(name="data", bufs=1))
        work = ctx.enter_context(tc.tile_pool(name="work", bufs=3))
        psA = ctx.enter_context(tc.tile_pool(name="psA", bufs=2, space="PSUM"))
        psK = ctx.enter_context(tc.tile_pool(name="psK", bufs=2, space="PSUM"))
        psV = ctx.enter_context(tc.tile_pool(name="psV", bufs=1, space="PSUM"))
        psM = ctx.enter_context(tc.tile_pool(name="psM", bufs=1, space="PSUM"))
        psN = ctx.enter_context(tc.tile_pool(name="psN", bufs=2, space="PSUM"))

        wt = consts.tile([D, BLOB_W], F32R, tag="wt")
        # weights split so the first conv terms unblock ASAP
        nc.gpsimd.dma_start(out=wt[:, 0:OFF_QKWL], in_=blob_d[:, 0:OFF_QKWL])
        nc.gpsimd.dma_start(out=wt[:, OFF_QKWL:OFF_GQH],
                            in_=blob_d[:, OFF_QKWL:OFF_GQH])
        nc.gpsimd.dma_start(out=wt[:, OFF_GQH:BLOB_W],
                            in_=blob_d[:, OFF_GQH:BLOB_W])
        bb = consts.tile([D, BBLOB_W], BF16, tag="bb")
        nc.gpsimd.dma_start(out=bb, in_=bblob_d[:])
        ones = consts.tile([D, 1], F32, tag="ones")
        nc.vector.memset(ones, 1.0)
        # x ships once (fp32); hi/lo/bf16 derived on device per quarter.
        xh_t = consts.tile([D, B, LP], F32R, tag="xh_t")
        xl_t = consts.tile([D, B, LP], F32R, tag="xl_t")
        xb_t = consts.tile([D, B, LP], BF16, tag="xb_t")
        qs = (0, 515, 1027, 1539, LP)
        for b in range(B):
            stage = data.tile([D, LP], F32, tag="xstage", name="xstage")
            for i in range(4):
                sl = slice(qs[i], qs[i + 1])
                nc.sync.dma_start(out=stage[:, sl],
                                  in_=xr_d[:, b * LP + qs[i]:b * LP + qs[i + 1]])
                nc.gpsimd.tensor_copy(xh_t[:, b, sl], stage[:, sl])
                nc.vector.scalar_tensor_tensor(
                    xl_t[:, b, sl], stage[:, sl], 0.0, xh_t[:, b, sl],
                    op0=ALU.add, op1=ALU.subtract)
                nc.gpsimd.tensor_copy(xb_t[:, b, sl], stage[:, sl])

        qkwh = wt[:, OFF_QKWH:OFF_QKWL].rearrange("p (a t d) -> p a t d", a=2, t=KS)
        qkwl = wt[:, OFF_QKWL:OFF_GQH].rearrange("p (a t d) -> p a t d", a=2, t=KS)
        gqh = wt[:, OFF_GQH:OFF_GQL].rearrange("p (a r) -> p a r", a=2)
        gql = wt[:, OFF_GQL:OFF_GKH].rearrange("p (a r) -> p a r", a=2)
        gkh = wt[:, OFF_GKH:OFF_GKL]
        gkl = wt[:, OFF_GKL:OFF_QKB]
        qkb = wt[:, OFF_QKB:OFF_QKB + 2]
        xh = [xh_t[:, 0, :], xh_t[:, 1, :]]
        xl = [xl_t[:, 0, :], xl_t[:, 1, :]]
        vw_b = bb[:, BOFF_VW:BOFF_VW + D]
        pw_b = bb[:, BOFF_PW:BOFF_PW + D]
        xb = [xb_t[:, 0, :], xb_t[:, 1, :]]

        # per-batch tiles
        t_qkh, t_qkl, t_phiq, t_phiqb, t_phik, t_phikb, t_vau, t_ost = (
            [], [], [], [], [], [], [], [])
        t_mt, t_pc, t_sk, t_rd = [], [], [], []
        for b in range(B):
            t_qkh.append(data.tile([D, 2, L], F32R, tag=f"qkh{b}", name=f"qkh{b}"))
            t_qkl.append(data.tile([D, 2, L], F32R, tag=f"qkl{b}", name=f"qkl{b}"))
            t_phiq.append(data.tile([R, L], F32, tag=f"phiq{b}", name=f"phiq{b}"))
            t_phiqb.append(data.tile([R, L], BF16, tag=f"phiqb{b}", name=f"phiqb{b}"))
            t_phik.append(data.tile([128, NT, R], F32, tag=f"phik{b}", name=f"phik{b}"))
            t_phikb.append(data.tile([128, NT, R], BF16, tag=f"phikb{b}", name=f"phikb{b}"))
            t_vau.append(data.tile([128, NT, D], BF16, tag=f"vau{b}", name=f"vau{b}"))
            t_ost.append(data.tile([128, NT, D], BF16, tag=f"ost{b}", name=f"ost{b}"))
            t_mt.append(data.tile([D, R], BF16, tag=f"mt{b}", name=f"mt{b}"))
            t_pc.append(data.tile([R, D], BF16, tag=f"pc{b}", name=f"pc{b}"))
            t_sk.append(data.tile([R, 1], F32, tag=f"sk{b}", name=f"sk{b}"))
            t_rd.append(data.tile([128, NT], F32, tag=f"rd{b}", name=f"rd{b}"))

        def conv(b):
            qk_h, qk_l = t_qkh[b], t_qkl[b]
            for p in range(2):
                for c in range(NCH):
                    ps = psA.tile([128, 512], F32, tag="psA")
                    first = True
                    for t in range(KS):
                        rh = xh[b][:, c * 512 + t:c * 512 + t + 512]
                        rl = xl[b][:, c * 512 + t:c * 512 + t + 512]
                        nc.tensor.matmul(ps, lhsT=qkwh[:, p, t, :], rhs=rh,
                                         start=first, stop=False)
                        first = False
                        nc.tensor.matmul(ps, lhsT=qkwh[:, p, t, :], rhs=rl,
                                         start=False, stop=False)
                        nc.tensor.matmul(ps, lhsT=qkwl[:, p, t, :], rhs=rh,
                                         start=False, stop=(t == KS - 1))
                    hs = qk_h[:, p, c * 512:(c + 1) * 512]
                    nc.scalar.add(hs, ps, qkb[:, p:p + 1])
                    nc.vector.scalar_tensor_tensor(
                        qk_l[:, p, c * 512:(c + 1) * 512], ps, qkb[:, p:p + 1],
                        hs, op0=ALU.add, op1=ALU.subtract)

        def sketch_q(b):
            qk_h, qk_l, phiq = t_qkh[b], t_qkl[b], t_phiq[b]
            for c in range(NCH):
                rh = qk_h[:, 0, c * 512:(c + 1) * 512]
                rl = qk_l[:, 0, c * 512:(c + 1) * 512]
                us = []
                for g in range(2):
                    u = psA.tile([128, 512], F32, tag="psA")
                    nc.tensor.matmul(u, lhsT=gqh[:, g, :], rhs=rh,
                                     start=True, stop=False)
                    nc.tensor.matmul(u, lhsT=gqh[:, g, :], rhs=rl,
                                     start=False, stop=False)
                    nc.tensor.matmul(u, lhsT=gql[:, g, :], rhs=rh,
                                     start=False, stop=True)
                    us.append(u)
                u1s = work.tile([128, 512], F32, tag="u1s")
                nc.scalar.copy(u1s, us[0])
                nc.vector.tensor_mul(phiq[:, c * 512:(c + 1) * 512], u1s, us[1])
            phiqb = t_phiqb[b]
            for hh in range(2):
                sl = slice(hh * (L // 2), (hh + 1) * (L // 2))
                nc.scalar.activation(phiq[:, sl], phiq[:, sl], AF.Tanh,
                                     scale=1.0 / SQRT_R)
                nc.gpsimd.tensor_copy(phiqb[:, sl], phiq[:, sl])

        def sketch_k(b):
            qk_h, qk_l, phik = t_qkh[b], t_qkl[b], t_phik[b]
            for mg in range(NT // 2):
                uu = psK.tile([128, 2, 256], F32, tag="uu")
                for j in range(2):
                    m = mg * 2 + j
                    klh = qk_h[:, 1, m * 128:(m + 1) * 128]
                    kll = qk_l[:, 1, m * 128:(m + 1) * 128]
                    nc.tensor.matmul(uu[:, j, :], lhsT=klh, rhs=gkh,
                                     start=(j == 0), stop=False,
                                     skip_group_check=True)
                    nc.tensor.matmul(uu[:, j, :], lhsT=kll, rhs=gkh,
                                     start=False, stop=False,
                                     skip_group_check=True)
                    nc.tensor.matmul(uu[:, j, :], lhsT=klh, rhs=gkl,
                                     start=False, stop=(j == 1),
                                     skip_group_check=True)
                u1k = work.tile([128, 2, 128], F32, tag="u1k")
                nc.scalar.copy(u1k, uu[:, :, 0:128])
                nc.vector.tensor_mul(phik[:, mg * 2:(mg + 1) * 2, :], u1k,
                                     uu[:, :, 128:256])
            pf = phik.rearrange("p a b -> p (a b)")
            pfb = t_phikb[b].rearrange("p a b -> p (a b)")
            for hh in range(2):
                sl = slice(hh * (NT // 2) * R, (hh + 1) * (NT // 2) * R)
                nc.scalar.activation(pf[:, sl], pf[:, sl], AF.Tanh,
                                     scale=1.0 / SQRT_R)
                nc.gpsimd.tensor_copy(pfb[:, sl], pf[:, sl])

        def vconv(b):
            vau = t_vau[b]
            for vg in range(NT // 4):
                vp = psV.tile([128, 4, D], F32, tag="vp")
                for j in range(4):
                    m = vg * 4 + j
                    nc.tensor.matmul(
                        vp[:, j, :],
                        lhsT=xb[b][:, KS - 1 + m * 128:KS - 1 + (m + 1) * 128],
                        rhs=vw_b, start=(j == 0), stop=(j == 3),
                        skip_group_check=True)
                nc.scalar.copy(vau[:, vg * 4:(vg + 1) * 4, :], vp)

        def finale(b):
            phiq, phiqb, phik, phikb = (t_phiq[b], t_phiqb[b], t_phik[b],
                                        t_phikb[b])
            vau, ost = t_vau[b], t_ost[b]
            mt_sb, pc_sb, sk_sb, rd = t_mt[b], t_pc[b], t_sk[b], t_rd[b]
            # M~^T [d, r] + s_k [r, 1] share one psum bank/group
            mtile = psM.tile([128, 512], F32, tag="psM")
            for m in range(NT):
                nc.tensor.matmul(mtile[:, 0:R], lhsT=vau[:, m, :],
                                 rhs=phikb[:, m, :], start=(m == 0),
                                 stop=False, skip_group_check=True)
            for m in range(NT):
                nc.tensor.matmul(mtile[:, R:R + 1], lhsT=phik[:, m, :],
                                 rhs=ones, start=False, stop=(m == NT - 1),
                                 skip_group_check=True)
            nc.scalar.copy(mt_sb, mtile[:, 0:R])
            nc.vector.tensor_copy(sk_sb, mtile[:, R:R + 1])
            # Pc [r, j] = M~ @ pw
            pcp = psM.tile([128, 512], F32, tag="psM")
            nc.tensor.matmul(pcp[:, 0:D], lhsT=mt_sb, rhs=pw_b,
                             start=True, stop=True, skip_group_check=True)
            nc.scalar.copy(pc_sb, pcp[:, 0:D])
            # den columns [m-tile, 1]
            dn = psM.tile([128, 512], F32, tag="psM")
            for m in range(NT):
                nc.tensor.matmul(dn[:, m:m + 1],
                                 lhsT=phiq[:, m * 128:(m + 1) * 128],
                                 rhs=sk_sb, start=(m == 0),
                                 stop=(m == NT - 1), skip_group_check=True)
            nc.vector.reciprocal(rd, dn[:, 0:NT])
            # num [m, j] groups of 4 + divide + ship
            for mg in range(NT // 4):
                nps = psN.tile([128, 4, D], F32, tag="psN")
                for j in range(4):
                    m = mg * 4 + j
                    nc.tensor.matmul(nps[:, j, :],
                                     lhsT=phiqb[:, m * 128:(m + 1) * 128],
                                     rhs=pc_sb, start=(j == 0),
                                     stop=(j == 3), skip_group_check=True)
                for j in range(4):
                    m = mg * 4 + j
                    if j % 2 == 0:
                        nc.vector.tensor_scalar(ost[:, m, :], nps[:, j, :],
                                                rd[:, m:m + 1], None,
                                                op0=ALU.mult)
                    else:
                        nc.scalar.activation(ost[:, m, :], nps[:, j, :],
                                             AF.Identity,
                                             scale=rd[:, m:m + 1])
                eng = nc.scalar if mg % 2 == 0 else nc.sync
                eng.dma_start(out=out_d[b, :, mg * 4:(mg + 1) * 4, :],
                              in_=ost[:, mg * 4:(mg + 1) * 4, :])

        # staggered emission: hide b0 finale latency under b1 conv
        conv(0)
        sketch_q(0)
        sketch_k(0)
        vconv(0)
        conv(1)
        finale(0)
        sketch_q(1)
        sketch_k(1)
        vconv(1)
        finale(1)
    nc.compile()
    return nc


def _split12(a):
    """Round-half-up split at 12 mantissa bits: a = hi + lo exactly, with hi
    representable in the PE's fp32r operand precision (probe-verified)."""
    a = np.ascontiguousarray(np.asarray(a, np.float32))
    u = a.view(np.uint32)
    hi = ((u + np.uint32(0x800)) & np.uint32(0xFFFFF000)).view(np.float32).copy()
    lo = (a - hi).astype(np.float32)
    return hi, lo


def _prep_in_maps(inputs):
    import ml_dtypes

    def f32(a):
        return np.ascontiguousarray(np.asarray(a), dtype=np.float32)

    x = f32(inputs["x"])
    q_w = f32(inputs["q_w"]).reshape(H, D, D, KS)
    k_w = f32(inputs["k_w"]).reshape(H, D, D, KS)
    v_w = f32(inputs["v_w"]).reshape(H, D, D)
    q_b = f32(inputs["q_b"]).reshape(H, D)
    k_b = f32(inputs["k_b"]).reshape(H, D)
    proj_w = f32(inputs["proj_w"])
    gq = float(np.asarray(inputs["gamma_q"]).reshape(-1)[0])
    bq = float(np.asarray(inputs["beta_q"]).reshape(-1)[0])
    gk = float(np.asarray(inputs["gamma_k"]).reshape(-1)[0])
    bk = float(np.asarray(inputs["beta_k"]).reshape(-1)[0])

    xp = np.zeros((D, B, LP), np.float32)
    xp[:, :, KS - 1:] = x.transpose(1, 0, 2)
    gq_s = np.stack([f32(inputs["g1_q"]), f32(inputs["g2_q"])], axis=1)  # [D,2,R]
    gk_s = np.concatenate([f32(inputs["g1_k"]), f32(inputs["g2_k"])], axis=1)
    gq_h, gq_l = _split12(gq_s)
    gk_h, gk_l = _split12(gk_s)

    in_maps = []
    for h in range(H):
        blob = np.zeros((D, BLOB_W), np.float32)
        wq_h, wq_l = _split12((gq * q_w[h]).transpose(1, 2, 0))  # [c, t, d]
        wk_h, wk_l = _split12((gk * k_w[h]).transpose(1, 2, 0))
        qh = blob[:, OFF_QKWH:OFF_QKWL].reshape(D, 2, KS, D)
        qh[:, 0], qh[:, 1] = wq_h, wk_h
        ql = blob[:, OFF_QKWL:OFF_GQH].reshape(D, 2, KS, D)
        ql[:, 0], ql[:, 1] = wq_l, wk_l
        blob[:, OFF_GQH:OFF_GQL] = gq_h.reshape(D, 2 * R)
        blob[:, OFF_GQL:OFF_GKH] = gq_l.reshape(D, 2 * R)
        blob[:, OFF_GKH:OFF_GKL] = gk_h
        blob[:, OFF_GKL:OFF_QKB] = gk_l
        blob[:, OFF_QKB] = gq * q_b[h] + bq
        blob[:, OFF_QKB + 1] = gk * k_b[h] + bk
        xraw = np.ascontiguousarray(xp.reshape(D, B * LP))
        bblob = np.empty((D, BBLOB_W), ml_dtypes.bfloat16)
        bblob[:, BOFF_VW:BOFF_VW + D] = v_w[h].T.astype(ml_dtypes.bfloat16)
        bblob[:, BOFF_PW:BOFF_PW + D] = (
            proj_w[:, h * D:(h + 1) * D].T.astype(ml_dtypes.bfloat16))
        in_maps.append(dict(blob=blob, xraw=xraw, bblob=bblob))
    return in_maps


def kernel(**inputs):
    global _built_nc, last_results
    if _built_nc is None:
        _built_nc = _build()
    in_maps = _prep_in_maps(inputs)
    res = run_bass_kernel_spmd(_built_nc, in_maps, list(range(H)))
    last_results = res
    parts = np.stack([np.asarray(res.results[c]["outp"], dtype=np.float32)
                      for c in range(H)])            # [H,B,128,NT,D]
    out = parts.sum(axis=0, dtype=np.float32)        # [B, 128(p), NT(m), D]
    out = out.transpose(0, 2, 1, 3).reshape(B, L, D)  # l = m*128 + p
    out = np.ascontiguousarray(out)
    out += np.asarray(inputs["proj_b"], np.float32)[None, None, :]
    return out.astype(np.float32)


# revision 14
# speedup vs baseline: 1.3376x; 1.0285x over previous
"""Conv1D-MHSA (sketched linear attention) Trainium2 kernel, v2.

Math (per batch b, head h; head h -> core h):
    q = conv1d_K3(x_pad, q_w) ; k likewise ; v = conv1d_K1(x, v_w)
    phi_q = tanh((q^T g1_q)*(q^T g2_q)/sqrt(R))  (phi_k likewise; the sqrt(R)
    post-scales cancel between numerator and denominator, eps -> ~0 dropped)
    out_h = diag(1/(phi_q^T s_k)) . phi_q^T (M~ pw),  M~ = phi_k^T v,
    s_k = colsum(phi_k);  host sums the 8 per-head partials + proj_b.

Precision: the denominator path (conv, sketches, phi_q, s_k, den) is
catastrophically cancellation-sensitive (operand rounding at even 2^-20
fails the 2e-2 budget), so those matmuls run as error-compensated fp32r
3-term products (hi/lo splits; hi*hi + hi*lo + lo*hi), which measures
bit-comparable to strict fp32 but streams at 1 cycle/row instead of 4.
The numerator path (v, M~, Pc=M~.pw, num) runs bf16.

Structure per (b, h): conv accumulates 9 fp32r taps/terms per 512-chunk in
PSUM; ACT evacuates hi (rounds to fp32r) + DVE computes lo. Sketches are
3-term fp32r. s_k is 16 one-column matmuls (rhs=ones), den is 16 one-column
matmuls (rhs=s_k) in [l,1] layout, so the divide becomes a per-partition
tensor_scalar and no L-wide broadcast matmul is needed. M~^T is accumulated
directly in [d, r] layout (lhsT=v tiles) so Pc = (M~^T)^T pw needs no
transpose. Batches are staggered (b1 conv emitted before b0 finale) to hide
the finale's elementwise latency under conv matmuls.
"""

import numpy as np
from contextlib import ExitStack

import concourse.bacc as bacc
import concourse.mybir as mybir
import concourse.tile as tile
from concourse.bass_utils import run_bass_kernel_spmd

F32 = mybir.dt.float32
F32R = mybir.dt.float32r
BF16 = mybir.dt.bfloat16
AF = mybir.ActivationFunctionType
ALU = mybir.AluOpType

B = 2
D = 128
L = 2048
H = 8
R = 128
KS = 3
LP = L + KS - 1
NCH = L // 512
NT = L // 128
SQRT_R = float(np.sqrt(R))

# f32r blob layout (free-dim offsets)
OFF_QKWH = 0                       # [2, 3, 128]
OFF_QKWL = OFF_QKWH + 2 * KS * D   # 768
OFF_GQH = OFF_QKWL + 2 * KS * D    # 1536: [2, 128]
OFF_GQL = OFF_GQH + 2 * R          # 1792
OFF_GKH = OFF_GQL + 2 * R          # 2048: [256] = [g1k | g2k]
OFF_GKL = OFF_GKH + 2 * R          # 2304
OFF_QKB = OFF_GKL + 2 * R          # 2560: [2]
BLOB_W = OFF_QKB + 2               # 2562 (weights only)
XR_W = B * LP                      # raw x, both batches
BOFF_VW = 0
BOFF_PW = BOFF_VW + D
BBLOB_W = BOFF_PW + D

_built_nc = None
last_results = None


def _build():
    nc = bacc.Bacc(None, target_bir_lowering=False)
    blob_d = nc.declare_dram_parameter("blob", [D, BLOB_W], F32R, isOutput=False)
    xr_d = nc.declare_dram_parameter("xraw", [D, XR_W], F32, isOutput=False)
    bblob_d = nc.declare_dram_parameter("bblob", [D, BBLOB_W], BF16, isOutput=False)
    out_d = nc.declare_dram_parameter("outp", [B, 128, NT, D], BF16, isOutput=True)

    with ExitStack() as ctx:
        tc = ctx.enter_context(tile.TileContext(nc))
        consts = ctx.enter_context(tc.tile_pool(name="consts", bufs=1))
        data = ctx.enter_context(tc.tile_pool(name="data", bufs=1))
        work = ctx.enter_context(tc.tile_pool(name="work", bufs=3))
        psA = ctx.enter_context(tc.tile_pool(name="psA", bufs=2, space="PSUM"))
        psK = ctx.enter_context(tc.tile_pool(name="psK", bufs=2, space="PSUM"))
        psV = ctx.enter_context(tc.tile_pool(name="psV", bufs=1, space="PSUM"))
        psM = ctx.enter_context(tc.tile_pool(name="psM", bufs=1, space="PSUM"))
        psN = ctx.enter_context(tc.tile_pool(name="psN", bufs=2, space="PSUM"))

        wt = consts.tile([D, BLOB_W], F32R, tag="wt")
        # weights split so the first conv terms unblock ASAP
        nc.gpsimd.dma_start(out=wt[:, 0:OFF_QKWL], in_=blob_d[:, 0:OFF_QKWL])
        nc.gpsimd.dma_start(out=wt[:, OFF_QKWL:OFF_GQH],
                            in_=blob_d[:, OFF_QKWL:OFF_GQH])
        nc.gpsimd.dma_start(out=wt[:, OFF_GQH:BLOB_W],
                            in_=blob_d[:, OFF_GQH:BLOB_W])
        bb = consts.tile([D, BBLOB_W], BF16, tag="bb")
        nc.gpsimd.dma_start(out=bb, in_=bblob_d[:])
        ones = consts.tile([D, 1], F32, tag="ones")
        nc.vector.memset(ones, 1.0)
        # x ships once (fp32); hi/lo/bf16 derived on device per quarter.
        xh_t = consts.tile([D, B, LP], F32R, tag="xh_t")
        xl_t = consts.tile([D, B, LP], F32R, tag="xl_t")
        xb_t = consts.tile([D, B, LP], BF16, tag="xb_t")
        qs = (0, 515, 1027, 1539, LP)
        for b in range(B):
            stage = data.tile([D, LP], F32, tag="xstage", name="xstage")
            for i in range(4):
                sl = slice(qs[i], qs[i + 1])
                nc.sync.dma_start(out=stage[:, sl],
                                  in_=xr_d[:, b * LP + qs[i]:b * LP + qs[i + 1]])
                nc.gpsimd.tensor_copy(xh_t[:, b, sl], stage[:, sl])
                nc.vector.scalar_tensor_tensor(
                    xl_t[:, b, sl], stage[:, sl], 0.0, xh_t[:, b, sl],
                    op0=ALU.add, op1=ALU.subtract)
                nc.gpsimd.tensor_copy(xb_t[:, b, sl], stage[:, sl])

        qkwh = wt[:, OFF_QKWH:OFF_QKWL].rearrange("p (a t d) -> p a t d", a=2, t=KS)
        qkwl = wt[:, OFF_QKWL:OFF_GQH].rearrange("p (a t d) -> p a t d", a=2, t=KS)
        gqh = wt[:, OFF_GQH:OFF_GQL].rearrange("p (a r) -> p a r", a=2)
        gql = wt[:, OFF_GQL:OFF_GKH].rearrange("p (a r) -> p a r", a=2)
        gkh = wt[:, OFF_GKH:OFF_GKL]
        gkl = wt[:, OFF_GKL:OFF_QKB]
        qkb = wt[:, OFF_QKB:OFF_QKB + 2]
        xh = [xh_t[:, 0, :], xh_t[:, 1, :]]
        xl = [xl_t[:, 0, :], xl_t[:, 1, :]]
        vw_b = bb[:, BOFF_VW:BOFF_VW + D]
        pw_b = bb[:, BOFF_PW:BOFF_PW + D]
        xb = [xb_t[:, 0, :], xb_t[:, 1, :]]

        # per-batch tiles
        t_qkh, t_qkl, t_phiq, t_phiqb, t_phik, t_phikb, t_vau, t_ost = (
            [], [], [], [], [], [], [], [])
        t_mt, t_pc, t_sk, t_rd = [], [], [], []
        for b in range(B):
            t_qkh.append(data.tile([D, 2, L], F32R, tag=f"qkh{b}", name=f"qkh{b}"))
            t_qkl.append(data.tile([D, 2, L], F32R, tag=f"qkl{b}", name=f"qkl{b}"))
            t_phiq.append(data.tile([R, L], F32, tag=f"phiq{b}", name=f"phiq{b}"))
            t_phiqb.append(data.tile([R, L], BF16, tag=f"phiqb{b}", name=f"phiqb{b}"))
            t_phik.append(data.tile([128, NT, R], F32, tag=f"phik{b}", name=f"phik{b}"))
            t_phikb.append(data.tile([128, NT, R], BF16, tag=f"phikb{b}", name=f"phikb{b}"))
            t_vau.append(data.tile([128, NT, D], BF16, tag=f"vau{b}", name=f"vau{b}"))
            t_ost.append(data.tile([128, NT, D], BF16, tag=f"ost{b}", name=f"ost{b}"))
            t_mt.append(data.tile([D, R], BF16, tag=f"mt{b}", name=f"mt{b}"))
            t_pc.append(data.tile([R, D], BF16, tag=f"pc{b}", name=f"pc{b}"))
            t_sk.append(data.tile([R, 1], F32, tag=f"sk{b}", name=f"sk{b}"))
            t_rd.append(data.tile([128, NT], F32, tag=f"rd{b}", name=f"rd{b}"))

        def conv(b):
            qk_h, qk_l = t_qkh[b], t_qkl[b]
            for c in range(NCH):
                for p in range(2):
                    ps = psA.tile([128, 512], F32, tag="psA")
                    first = True
                    for t in range(KS):
                        rh = xh[b][:, c * 512 + t:c * 512 + t + 512]
                        rl = xl[b][:, c * 512 + t:c * 512 + t + 512]
                        nc.tensor.matmul(ps, lhsT=qkwh[:, p, t, :], rhs=rh,
                                         start=first, stop=False)
                        first = False
                        nc.tensor.matmul(ps, lhsT=qkwh[:, p, t, :], rhs=rl,
                                         start=False, stop=False)
                        nc.tensor.matmul(ps, lhsT=qkwl[:, p, t, :], rhs=rh,
                                         start=False, stop=(t == KS - 1))
                    hs = qk_h[:, p, c * 512:(c + 1) * 512]
                    nc.scalar.add(hs, ps, qkb[:, p:p + 1])
                    nc.vector.scalar_tensor_tensor(
                        qk_l[:, p, c * 512:(c + 1) * 512], ps, qkb[:, p:p + 1],
                        hs, op0=ALU.add, op1=ALU.subtract)

        def sketch_q(b):
            qk_h, qk_l, phiq = t_qkh[b], t_qkl[b], t_phiq[b]
            for c in range(NCH):
                rh = qk_h[:, 0, c * 512:(c + 1) * 512]
                rl = qk_l[:, 0, c * 512:(c + 1) * 512]
                us = []
                for g in range(2):
                    u = psA.tile([128, 512], F32, tag="psA")
                    nc.tensor.matmul(u, lhsT=gqh[:, g, :], rhs=rh,
                                     start=True, stop=False)
                    nc.tensor.matmul(u, lhsT=gqh[:, g, :], rhs=rl,
                                     start=False, stop=False)
                    nc.tensor.matmul(u, lhsT=gql[:, g, :], rhs=rh,
                                     start=False, stop=True)
                    us.append(u)
                u1s = work.tile([128, 512], F32, tag="u1s")
                nc.scalar.copy(u1s, us[0])
                nc.vector.tensor_mul(phiq[:, c * 512:(c + 1) * 512], u1s, us[1])
            phiqb = t_phiqb[b]
            for hh in range(2):
                sl = slice(hh * (L // 2), (hh + 1) * (L // 2))
                nc.scalar.activation(phiq[:, sl], phiq[:, sl], AF.Tanh,
                                     scale=1.0 / SQRT_R)
                nc.gpsimd.tensor_copy(phiqb[:, sl], phiq[:, sl])

        def sketch_k(b):
            qk_h, qk_l, phik = t_qkh[b], t_qkl[b], t_phik[b]
            for mg in range(NT // 2):
                uu = psK.tile([128, 2, 256], F32, tag="uu")
                for j in range(2):
                    m = mg * 2 + j
                    klh = qk_h[:, 1, m * 128:(m + 1) * 128]
                    kll = qk_l[:, 1, m * 128:(m + 1) * 128]
                    nc.tensor.matmul(uu[:, j, :], lhsT=klh, rhs=gkh,
                                     start=(j == 0), stop=False,
                                     skip_group_check=True)
                    nc.tensor.matmul(uu[:, j, :], lhsT=kll, rhs=gkh,
                                     start=False, stop=False,
                                     skip_group_check=True)
                    nc.tensor.matmul(uu[:, j, :], lhsT=klh, rhs=gkl,
                                     start=False, stop=(j == 1),
                                     skip_group_check=True)
                u1k = work.tile([128, 2, 128], F32, tag="u1k")
                nc.scalar.copy(u1k, uu[:, :, 0:128])
                nc.vector.tensor_mul(phik[:, mg * 2:(mg + 1) * 2, :], u1k,
                                     uu[:, :, 128:256])
            pf = phik.rearrange("p a b -> p (a b)")
            pfb = t_phikb[b].rearrange("p a b -> p (a b)")
            for hh in range(2):
                sl = slice(hh * (NT // 2) * R, (hh + 1) * (NT // 2) * R)
                nc.scalar.activation(pf[:, sl], pf[:, sl], AF.Tanh,
                                     scale=1.0 / SQRT_R)
                nc.gpsimd.tensor_copy(pfb[:, sl], pf[:, sl])

        def vconv(b):
            vau = t_vau[b]
            for vg in range(NT // 4):
                vp = psV.tile([128, 4, D], F32, tag="vp")
                for j in range(4):
                    m = vg * 4 + j
                    nc.tensor.matmul(
                        vp[:, j, :],
                        lhsT=xb[b][:, KS - 1 + m * 128:KS - 1 + (m + 1) * 128],
                        rhs=vw_b, start=(j == 0), stop=(j == 3),
                        skip_group_check=True)
                nc.scalar.copy(vau[:, vg * 4:(vg + 1) * 4, :], vp)

        def finale_a(b):
            phik, phikb, vau = t_phik[b], t_phikb[b], t_vau[b]
            mt_sb, pc_sb, sk_sb = t_mt[b], t_pc[b], t_sk[b]
            # M~^T [d, r] + s_k [r, 1] share one psum bank/group
            mtile = psM.tile([128, 512], F32, tag="psM")
            for m in range(NT):
                nc.tensor.matmul(mtile[:, 0:R], lhsT=vau[:, m, :],
                                 rhs=phikb[:, m, :], start=(m == 0),
                                 stop=False, skip_group_check=True)
            for m in range(NT):
                nc.tensor.matmul(mtile[:, R:R + 1], lhsT=phik[:, m, :],
                                 rhs=ones, start=False, stop=(m == NT - 1),
                                 skip_group_check=True)
            nc.scalar.copy(mt_sb, mtile[:, 0:R])
            nc.vector.tensor_copy(sk_sb, mtile[:, R:R + 1])
            # Pc [r, j] = M~ @ pw
            pcp = psM.tile([128, 512], F32, tag="psM")
            nc.tensor.matmul(pcp[:, 0:D], lhsT=mt_sb, rhs=pw_b,
                             start=True, stop=True, skip_group_check=True)
            nc.scalar.copy(pc_sb, pcp[:, 0:D])

        def finale_b(b):
            phiq, phiqb, ost = t_phiq[b], t_phiqb[b], t_ost[b]
            pc_sb, sk_sb, rd = t_pc[b], t_sk[b], t_rd[b]
            # den columns [m-tile, 1]
            dn = psM.tile([128, 512], F32, tag="psM")
            for m in range(NT):
                nc.tensor.matmul(dn[:, m:m + 1],
                                 lhsT=phiq[:, m * 128:(m + 1) * 128],
                                 rhs=sk_sb, start=(m == 0),
                                 stop=(m == NT - 1), skip_group_check=True)
            nc.vector.reciprocal(rd, dn[:, 0:NT])
            # num [m, j] groups of 4 + divide + ship
            for mg in range(NT // 4):
                nps = psN.tile([128, 4, D], F32, tag="psN")
                for j in range(4):
                    m = mg * 4 + j
                    nc.tensor.matmul(nps[:, j, :],
                                     lhsT=phiqb[:, m * 128:(m + 1) * 128],
                                     rhs=pc_sb, start=(j == 0),
                                     stop=(j == 3), skip_group_check=True)
                for j in range(4):
                    m = mg * 4 + j
                    if j % 2 == 0:
                        nc.vector.tensor_scalar(ost[:, m, :], nps[:, j, :],
                                                rd[:, m:m + 1], None,
                                                op0=ALU.mult)
                    else:
                        nc.scalar.activation(ost[:, m, :], nps[:, j, :],
                                             AF.Identity,
                                             scale=rd[:, m:m + 1])
                eng = nc.scalar if mg % 2 == 0 else nc.sync
                eng.dma_start(out=out_d[b, :, mg * 4:(mg + 1) * 4, :],
                              in_=ost[:, mg * 4:(mg + 1) * 4, :])

        # staggered emission: phik-side finale chain (A) hides under
        # sketch_q; b0's den/num stage (B) hides under b1's conv
        conv(0)
        sketch_k(0)
        vconv(0)
        sketch_q(0)
        finale_a(0)
        conv(1)
        finale_b(0)
        sketch_k(1)
        vconv(1)
        sketch_q(1)
        finale_a(1)
        finale_b(1)
    nc.compile()
    return nc


def _split12(a):
    """Round-half-up split at 12 mantissa bits: a = hi + lo exactly, with hi
    representable in the PE's fp32r operand precision (probe-verified)."""
    a = np.ascontiguousarray(np.asarray(a, np.float32))
    u = a.view(np.uint32)
    hi = ((u + np.uint32(0x800)) & np.uint32(0xFFFFF000)).view(np.float32).copy()
    lo = (a - hi).astype(np.float32)
    return hi, lo


def _prep_in_maps(inputs):
    import ml_dtypes

    def f32(a):
        return np.ascontiguousarray(np.asarray(a), dtype=np.float32)

    x = f32(inputs["x"])
    q_w = f32(inputs["q_w"]).reshape(H, D, D, KS)
    k_w = f32(inputs["k_w"]).reshape(H, D, D, KS)
    v_w = f32(inputs["v_w"]).reshape(H, D, D)
    q_b = f32(inputs["q_b"]).reshape(H, D)
    k_b = f32(inputs["k_b"]).reshape(H, D)
    proj_w = f32(inputs["proj_w"])
    gq = float(np.asarray(inputs["gamma_q"]).reshape(-1)[0])
    bq = float(np.asarray(inputs["beta_q"]).reshape(-1)[0])
    gk = float(np.asarray(inputs["gamma_k"]).reshape(-1)[0])
    bk = float(np.asarray(inputs["beta_k"]).reshape(-1)[0])

    xp = np.zeros((D, B, LP), np.float32)
    xp[:, :, KS - 1:] = x.transpose(1, 0, 2)
    gq_s = np.stack([f32(inputs["g1_q"]), f32(inputs["g2_q"])], axis=1)  # [D,2,R]
    gk_s = np.concatenate([f32(inputs["g1_k"]), f32(inputs["g2_k"])], axis=1)
    gq_h, gq_l = _split12(gq_s)
    gk_h, gk_l = _split12(gk_s)

    in_maps = []
    for h in range(H):
        blob = np.zeros((D, BLOB_W), np.float32)
        wq_h, wq_l = _split12((gq * q_w[h]).transpose(1, 2, 0))  # [c, t, d]
        wk_h, wk_l = _split12((gk * k_w[h]).transpose(1, 2, 0))
        qh = blob[:, OFF_QKWH:OFF_QKWL].reshape(D, 2, KS, D)
        qh[:, 0], qh[:, 1] = wq_h, wk_h
        ql = blob[:, OFF_QKWL:OFF_GQH].reshape(D, 2, KS, D)
        ql[:, 0], ql[:, 1] = wq_l, wk_l
        blob[:, OFF_GQH:OFF_GQL] = gq_h.reshape(D, 2 * R)
        blob[:, OFF_GQL:OFF_GKH] = gq_l.reshape(D, 2 * R)
        blob[:, OFF_GKH:OFF_GKL] = gk_h
        blob[:, OFF_GKL:OFF_QKB] = gk_l
        blob[:, OFF_QKB] = gq * q_b[h] + bq
        blob[:, OFF_QKB + 1] = gk * k_b[h] + bk
        xraw = np.ascontiguousarray(xp.reshape(D, B * LP))
        bblob = np.empty((D, BBLOB_W), ml_dtypes.bfloat16)
        bblob[:, BOFF_VW:BOFF_VW + D] = v_w[h].T.astype(ml_dtypes.bfloat16)
        bblob[:, BOFF_PW:BOFF_PW + D] = (
            proj_w[:, h * D:(h + 1) * D].T.astype(ml_dtypes.bfloat16))
        in_maps.append(dict(blob=blob, xraw=xraw, bblob=bblob))
    return in_maps


def kernel(**inputs):
    global _built_nc, last_results
    if _built_nc is None:
        _built_nc = _build()
    in_maps = _prep_in_maps(inputs)
    res = run_bass_kernel_spmd(_built_nc, in_maps, list(range(H)))
    last_results = res
    parts = np.stack([np.asarray(res.results[c]["outp"], dtype=np.float32)
                      for c in range(H)])            # [H,B,128,NT,D]
    out = parts.sum(axis=0, dtype=np.float32)        # [B, 128(p), NT(m), D]
    out = out.transpose(0, 2, 1, 3).reshape(B, L, D)  # l = m*128 + p
    out = np.ascontiguousarray(out)
    out += np.asarray(inputs["proj_b"], np.float32)[None, None, :]
    return out.astype(np.float32)


# revision 15
# speedup vs baseline: 1.3718x; 1.0255x over previous
"""Conv1D-MHSA (sketched linear attention) Trainium2 kernel, v2.

Math (per batch b, head h; head h -> core h):
    q = conv1d_K3(x_pad, q_w) ; k likewise ; v = conv1d_K1(x, v_w)
    phi_q = tanh((q^T g1_q)*(q^T g2_q)/sqrt(R))  (phi_k likewise; the sqrt(R)
    post-scales cancel between numerator and denominator, eps -> ~0 dropped)
    out_h = diag(1/(phi_q^T s_k)) . phi_q^T (M~ pw),  M~ = phi_k^T v,
    s_k = colsum(phi_k);  host sums the 8 per-head partials + proj_b.

Precision: the denominator path (conv, sketches, phi_q, s_k, den) is
catastrophically cancellation-sensitive (operand rounding at even 2^-20
fails the 2e-2 budget), so those matmuls run as error-compensated fp32r
3-term products (hi/lo splits; hi*hi + hi*lo + lo*hi), which measures
bit-comparable to strict fp32 but streams at 1 cycle/row instead of 4.
The numerator path (v, M~, Pc=M~.pw, num) runs bf16.

Structure per (b, h): conv accumulates 9 fp32r taps/terms per 512-chunk in
PSUM; ACT evacuates hi (rounds to fp32r) + DVE computes lo. Sketches are
3-term fp32r. s_k is 16 one-column matmuls (rhs=ones), den is 16 one-column
matmuls (rhs=s_k) in [l,1] layout, so the divide becomes a per-partition
tensor_scalar and no L-wide broadcast matmul is needed. M~^T is accumulated
directly in [d, r] layout (lhsT=v tiles) so Pc = (M~^T)^T pw needs no
transpose. Batches are staggered (b1 conv emitted before b0 finale) to hide
the finale's elementwise latency under conv matmuls.
"""

import numpy as np
from contextlib import ExitStack

import concourse.bacc as bacc
import concourse.mybir as mybir
import concourse.tile as tile
from concourse.bass_utils import run_bass_kernel_spmd

F32 = mybir.dt.float32
F32R = mybir.dt.float32r
BF16 = mybir.dt.bfloat16
AF = mybir.ActivationFunctionType
ALU = mybir.AluOpType

B = 2
D = 128
L = 2048
H = 8
R = 128
KS = 3
LP = L + KS - 1
NCH = L // 512
NT = L // 128
SQRT_R = float(np.sqrt(R))

# f32r blob layout (free-dim offsets)
OFF_QKWH = 0                       # [2, 3, 128]
OFF_QKWL = OFF_QKWH + 2 * KS * D   # 768
OFF_GQH = OFF_QKWL + 2 * KS * D    # 1536: [2, 128]
OFF_GQL = OFF_GQH + 2 * R          # 1792
OFF_GKH = OFF_GQL + 2 * R          # 2048: [256] = [g1k | g2k]
OFF_GKL = OFF_GKH + 2 * R          # 2304
OFF_QKB = OFF_GKL + 2 * R          # 2560: [2]
BLOB_W = OFF_QKB + 2               # 2562 (weights only)
XR_W = B * LP                      # raw x, both batches
BOFF_VW = 0
BOFF_PW = BOFF_VW + D
BBLOB_W = BOFF_PW + D

_built_nc = None
last_results = None


def _build():
    nc = bacc.Bacc(None, target_bir_lowering=False)
    blob_d = nc.declare_dram_parameter("blob", [D, BLOB_W], F32R, isOutput=False)
    xr_d = nc.declare_dram_parameter("xraw", [D, XR_W], F32, isOutput=False)
    bblob_d = nc.declare_dram_parameter("bblob", [D, BBLOB_W], BF16, isOutput=False)
    out_d = nc.declare_dram_parameter("outp", [B, 128, NT, D], BF16, isOutput=True)

    with ExitStack() as ctx:
        tc = ctx.enter_context(tile.TileContext(nc))
        consts = ctx.enter_context(tc.tile_pool(name="consts", bufs=1))
        data = ctx.enter_context(tc.tile_pool(name="data", bufs=1))
        work = ctx.enter_context(tc.tile_pool(name="work", bufs=3))
        psA = ctx.enter_context(tc.tile_pool(name="psA", bufs=2, space="PSUM"))
        psK = ctx.enter_context(tc.tile_pool(name="psK", bufs=2, space="PSUM"))
        psV = ctx.enter_context(tc.tile_pool(name="psV", bufs=1, space="PSUM"))
        psM = ctx.enter_context(tc.tile_pool(name="psM", bufs=1, space="PSUM"))
        psN = ctx.enter_context(tc.tile_pool(name="psN", bufs=2, space="PSUM"))

        wt = consts.tile([D, BLOB_W], F32R, tag="wt")
        # weights split so the first conv terms unblock ASAP
        nc.gpsimd.dma_start(out=wt[:, 0:OFF_QKWL], in_=blob_d[:, 0:OFF_QKWL])
        nc.gpsimd.dma_start(out=wt[:, OFF_QKWL:OFF_GQH],
                            in_=blob_d[:, OFF_QKWL:OFF_GQH])
        nc.gpsimd.dma_start(out=wt[:, OFF_GQH:BLOB_W],
                            in_=blob_d[:, OFF_GQH:BLOB_W])
        bb = consts.tile([D, BBLOB_W], BF16, tag="bb")
        nc.gpsimd.dma_start(out=bb, in_=bblob_d[:])
        ones = consts.tile([D, 1], F32, tag="ones")
        nc.vector.memset(ones, 1.0)
        # x ships once (fp32); hi/lo/bf16 derived on device per quarter.
        xh_t = consts.tile([D, B, LP], F32R, tag="xh_t")
        xl_t = consts.tile([D, B, LP], F32R, tag="xl_t")
        xb_t = consts.tile([D, B, LP], BF16, tag="xb_t")
        qs = (0, 515, 1027, 1539, LP)
        for b in range(B):
            stage = data.tile([D, LP], F32, tag="xstage", name="xstage")
            for i in range(4):
                sl = slice(qs[i], qs[i + 1])
                nc.sync.dma_start(out=stage[:, sl],
                                  in_=xr_d[:, b * LP + qs[i]:b * LP + qs[i + 1]])
                nc.gpsimd.tensor_copy(xh_t[:, b, sl], stage[:, sl])
                nc.vector.scalar_tensor_tensor(
                    xl_t[:, b, sl], stage[:, sl], 0.0, xh_t[:, b, sl],
                    op0=ALU.add, op1=ALU.subtract)
                nc.gpsimd.tensor_copy(xb_t[:, b, sl], stage[:, sl])

        qkwh = wt[:, OFF_QKWH:OFF_QKWL].rearrange("p (a t d) -> p a t d", a=2, t=KS)
        qkwl = wt[:, OFF_QKWL:OFF_GQH].rearrange("p (a t d) -> p a t d", a=2, t=KS)
        gqh = wt[:, OFF_GQH:OFF_GQL].rearrange("p (a r) -> p a r", a=2)
        gql = wt[:, OFF_GQL:OFF_GKH].rearrange("p (a r) -> p a r", a=2)
        gkh = wt[:, OFF_GKH:OFF_GKL]
        gkl = wt[:, OFF_GKL:OFF_QKB]
        qkb = wt[:, OFF_QKB:OFF_QKB + 2]
        xh = [xh_t[:, 0, :], xh_t[:, 1, :]]
        xl = [xl_t[:, 0, :], xl_t[:, 1, :]]
        vw_b = bb[:, BOFF_VW:BOFF_VW + D]
        pw_b = bb[:, BOFF_PW:BOFF_PW + D]
        xb = [xb_t[:, 0, :], xb_t[:, 1, :]]

        # per-batch tiles
        t_qkh, t_qkl, t_phiq, t_phiqb, t_phik, t_phikb, t_vau, t_ost = (
            [], [], [], [], [], [], [], [])
        t_mt, t_pc, t_sk, t_rd = [], [], [], []
        for b in range(B):
            t_qkh.append(data.tile([D, 2, L], F32R, tag=f"qkh{b}", name=f"qkh{b}"))
            t_qkl.append(data.tile([D, 2, L], F32R, tag=f"qkl{b}", name=f"qkl{b}"))
            t_phiq.append(data.tile([R, L], F32, tag=f"phiq{b}", name=f"phiq{b}"))
            t_phiqb.append(data.tile([R, L], BF16, tag=f"phiqb{b}", name=f"phiqb{b}"))
            t_phik.append(data.tile([128, NT, R], F32, tag=f"phik{b}", name=f"phik{b}"))
            t_phikb.append(data.tile([128, NT, R], BF16, tag=f"phikb{b}", name=f"phikb{b}"))
            t_vau.append(data.tile([128, NT, D], BF16, tag=f"vau{b}", name=f"vau{b}"))
            t_ost.append(data.tile([128, NT, D], BF16, tag=f"ost{b}", name=f"ost{b}"))
            t_mt.append(data.tile([D, R], BF16, tag=f"mt{b}", name=f"mt{b}"))
            t_pc.append(data.tile([R, D], BF16, tag=f"pc{b}", name=f"pc{b}"))
            t_sk.append(data.tile([R, 1], F32, tag=f"sk{b}", name=f"sk{b}"))
            t_rd.append(data.tile([128, NT], F32, tag=f"rd{b}", name=f"rd{b}"))

        def conv(b):
            qk_h, qk_l = t_qkh[b], t_qkl[b]
            for c in range(NCH):
                for p in range(2):
                    ps = psA.tile([128, 512], F32, tag="psA")
                    first = True
                    for t in range(KS):
                        rh = xh[b][:, c * 512 + t:c * 512 + t + 512]
                        rl = xl[b][:, c * 512 + t:c * 512 + t + 512]
                        nc.tensor.matmul(ps, lhsT=qkwh[:, p, t, :], rhs=rh,
                                         start=first, stop=False)
                        first = False
                        nc.tensor.matmul(ps, lhsT=qkwh[:, p, t, :], rhs=rl,
                                         start=False, stop=False)
                        nc.tensor.matmul(ps, lhsT=qkwl[:, p, t, :], rhs=rh,
                                         start=False, stop=(t == KS - 1))
                    hs = qk_h[:, p, c * 512:(c + 1) * 512]
                    nc.scalar.add(hs, ps, qkb[:, p:p + 1])
                    nc.vector.scalar_tensor_tensor(
                        qk_l[:, p, c * 512:(c + 1) * 512], ps, qkb[:, p:p + 1],
                        hs, op0=ALU.add, op1=ALU.subtract)

        def sketch_q(b):
            qk_h, qk_l, phiq = t_qkh[b], t_qkl[b], t_phiq[b]
            for c in range(NCH):
                rh = qk_h[:, 0, c * 512:(c + 1) * 512]
                rl = qk_l[:, 0, c * 512:(c + 1) * 512]
                us = []
                for g in range(2):
                    u = psA.tile([128, 512], F32, tag="psA")
                    nc.tensor.matmul(u, lhsT=gqh[:, g, :], rhs=rh,
                                     start=True, stop=False)
                    nc.tensor.matmul(u, lhsT=gqh[:, g, :], rhs=rl,
                                     start=False, stop=False)
                    nc.tensor.matmul(u, lhsT=gql[:, g, :], rhs=rh,
                                     start=False, stop=True)
                    us.append(u)
                u1s = work.tile([128, 512], F32, tag="u1s")
                nc.scalar.copy(u1s, us[0])
                nc.vector.tensor_mul(phiq[:, c * 512:(c + 1) * 512], u1s, us[1])
            phiqb = t_phiqb[b]
            for hh in range(2):
                sl = slice(hh * (L // 2), (hh + 1) * (L // 2))
                nc.scalar.activation(phiq[:, sl], phiq[:, sl], AF.Tanh,
                                     scale=1.0 / SQRT_R)
                nc.gpsimd.tensor_copy(phiqb[:, sl], phiq[:, sl])

        def sketch_k(b):
            qk_h, qk_l, phik = t_qkh[b], t_qkl[b], t_phik[b]
            for mg in range(NT // 2):
                uu = psK.tile([128, 2, 256], F32, tag="uu")
                for j in range(2):
                    m = mg * 2 + j
                    klh = qk_h[:, 1, m * 128:(m + 1) * 128]
                    kll = qk_l[:, 1, m * 128:(m + 1) * 128]
                    nc.tensor.matmul(uu[:, j, :], lhsT=klh, rhs=gkh,
                                     start=(j == 0), stop=False,
                                     skip_group_check=True)
                    nc.tensor.matmul(uu[:, j, :], lhsT=kll, rhs=gkh,
                                     start=False, stop=False,
                                     skip_group_check=True)
                    nc.tensor.matmul(uu[:, j, :], lhsT=klh, rhs=gkl,
                                     start=False, stop=(j == 1),
                                     skip_group_check=True)
                u1k = work.tile([128, 2, 128], F32, tag="u1k")
                nc.vector.tensor_copy(u1k, uu[:, :, 0:128])
                nc.vector.tensor_mul(phik[:, mg * 2:(mg + 1) * 2, :], u1k,
                                     uu[:, :, 128:256])
            pf = phik.rearrange("p a b -> p (a b)")
            pfb = t_phikb[b].rearrange("p a b -> p (a b)")
            for hh in range(2):
                sl = slice(hh * (NT // 2) * R, (hh + 1) * (NT // 2) * R)
                nc.scalar.activation(pf[:, sl], pf[:, sl], AF.Tanh,
                                     scale=1.0 / SQRT_R)
                nc.gpsimd.tensor_copy(pfb[:, sl], pf[:, sl])

        def vconv(b):
            vau = t_vau[b]
            for vg in range(NT // 4):
                vp = psV.tile([128, 4, D], F32, tag="vp")
                for j in range(4):
                    m = vg * 4 + j
                    nc.tensor.matmul(
                        vp[:, j, :],
                        lhsT=xb[b][:, KS - 1 + m * 128:KS - 1 + (m + 1) * 128],
                        rhs=vw_b, start=(j == 0), stop=(j == 3),
                        skip_group_check=True)
                eng = nc.scalar if vg % 2 == 0 else nc.vector
                if vg % 2 == 0:
                    nc.scalar.copy(vau[:, vg * 4:(vg + 1) * 4, :], vp)
                else:
                    nc.vector.tensor_copy(vau[:, vg * 4:(vg + 1) * 4, :], vp)

        def finale_a(b):
            phik, phikb, vau = t_phik[b], t_phikb[b], t_vau[b]
            mt_sb, pc_sb, sk_sb = t_mt[b], t_pc[b], t_sk[b]
            # M~^T [d, r] + s_k [r, 1] share one psum bank/group
            mtile = psM.tile([128, 512], F32, tag="psM")
            for m in range(NT):
                nc.tensor.matmul(mtile[:, 0:R], lhsT=vau[:, m, :],
                                 rhs=phikb[:, m, :], start=(m == 0),
                                 stop=False, skip_group_check=True)
            for m in range(NT):
                nc.tensor.matmul(mtile[:, R:R + 1], lhsT=phik[:, m, :],
                                 rhs=ones, start=False, stop=(m == NT - 1),
                                 skip_group_check=True)
            nc.scalar.copy(mt_sb, mtile[:, 0:R])
            nc.vector.tensor_copy(sk_sb, mtile[:, R:R + 1])
            # Pc [r, j] = M~ @ pw
            pcp = psM.tile([128, 512], F32, tag="psM")
            nc.tensor.matmul(pcp[:, 0:D], lhsT=mt_sb, rhs=pw_b,
                             start=True, stop=True, skip_group_check=True)
            nc.scalar.copy(pc_sb, pcp[:, 0:D])

        def finale_b(b):
            phiq, phiqb, ost = t_phiq[b], t_phiqb[b], t_ost[b]
            pc_sb, sk_sb, rd = t_pc[b], t_sk[b], t_rd[b]
            # den columns [m-tile, 1]
            dn = psM.tile([128, 512], F32, tag="psM")
            for m in range(NT):
                nc.tensor.matmul(dn[:, m:m + 1],
                                 lhsT=phiq[:, m * 128:(m + 1) * 128],
                                 rhs=sk_sb, start=(m == 0),
                                 stop=(m == NT - 1), skip_group_check=True)
            nc.vector.reciprocal(rd, dn[:, 0:NT])
            # num [m, j] groups of 4 + divide + ship
            for mg in range(NT // 4):
                nps = psN.tile([128, 4, D], F32, tag="psN")
                for j in range(4):
                    m = mg * 4 + j
                    nc.tensor.matmul(nps[:, j, :],
                                     lhsT=phiqb[:, m * 128:(m + 1) * 128],
                                     rhs=pc_sb, start=(j == 0),
                                     stop=(j == 3), skip_group_check=True)
                for j in range(4):
                    m = mg * 4 + j
                    if j % 2 == 0:
                        nc.vector.tensor_scalar(ost[:, m, :], nps[:, j, :],
                                                rd[:, m:m + 1], None,
                                                op0=ALU.mult)
                    else:
                        nc.scalar.activation(ost[:, m, :], nps[:, j, :],
                                             AF.Identity,
                                             scale=rd[:, m:m + 1])
                eng = nc.scalar if mg % 2 == 0 else nc.sync
                eng.dma_start(out=out_d[b, :, mg * 4:(mg + 1) * 4, :],
                              in_=ost[:, mg * 4:(mg + 1) * 4, :])

        # staggered emission: phik-side finale chain (A) hides under
        # sketch_q; b0's den/num stage (B) hides under b1's conv
        conv(0)
        sketch_k(0)
        vconv(0)
        sketch_q(0)
        finale_a(0)
        conv(1)
        finale_b(0)
        sketch_k(1)
        vconv(1)
        sketch_q(1)
        finale_a(1)
        finale_b(1)
    nc.compile()
    return nc


def _split12(a):
    """Round-half-up split at 12 mantissa bits: a = hi + lo exactly, with hi
    representable in the PE's fp32r operand precision (probe-verified)."""
    a = np.ascontiguousarray(np.asarray(a, np.float32))
    u = a.view(np.uint32)
    hi = ((u + np.uint32(0x800)) & np.uint32(0xFFFFF000)).view(np.float32).copy()
    lo = (a - hi).astype(np.float32)
    return hi, lo


def _prep_in_maps(inputs):
    import ml_dtypes

    def f32(a):
        return np.ascontiguousarray(np.asarray(a), dtype=np.float32)

    x = f32(inputs["x"])
    q_w = f32(inputs["q_w"]).reshape(H, D, D, KS)
    k_w = f32(inputs["k_w"]).reshape(H, D, D, KS)
    v_w = f32(inputs["v_w"]).reshape(H, D, D)
    q_b = f32(inputs["q_b"]).reshape(H, D)
    k_b = f32(inputs["k_b"]).reshape(H, D)
    proj_w = f32(inputs["proj_w"])
    gq = float(np.asarray(inputs["gamma_q"]).reshape(-1)[0])
    bq = float(np.asarray(inputs["beta_q"]).reshape(-1)[0])
    gk = float(np.asarray(inputs["gamma_k"]).reshape(-1)[0])
    bk = float(np.asarray(inputs["beta_k"]).reshape(-1)[0])

    xp = np.zeros((D, B, LP), np.float32)
    xp[:, :, KS - 1:] = x.transpose(1, 0, 2)
    gq_s = np.stack([f32(inputs["g1_q"]), f32(inputs["g2_q"])], axis=1)  # [D,2,R]
    gk_s = np.concatenate([f32(inputs["g1_k"]), f32(inputs["g2_k"])], axis=1)
    gq_h, gq_l = _split12(gq_s)
    gk_h, gk_l = _split12(gk_s)

    in_maps = []
    for h in range(H):
        blob = np.zeros((D, BLOB_W), np.float32)
        wq_h, wq_l = _split12((gq * q_w[h]).transpose(1, 2, 0))  # [c, t, d]
        wk_h, wk_l = _split12((gk * k_w[h]).transpose(1, 2, 0))
        qh = blob[:, OFF_QKWH:OFF_QKWL].reshape(D, 2, KS, D)
        qh[:, 0], qh[:, 1] = wq_h, wk_h
        ql = blob[:, OFF_QKWL:OFF_GQH].reshape(D, 2, KS, D)
        ql[:, 0], ql[:, 1] = wq_l, wk_l
        blob[:, OFF_GQH:OFF_GQL] = gq_h.reshape(D, 2 * R)
        blob[:, OFF_GQL:OFF_GKH] = gq_l.reshape(D, 2 * R)
        blob[:, OFF_GKH:OFF_GKL] = gk_h
        blob[:, OFF_GKL:OFF_QKB] = gk_l
        blob[:, OFF_QKB] = gq * q_b[h] + bq
        blob[:, OFF_QKB + 1] = gk * k_b[h] + bk
        xraw = np.ascontiguousarray(xp.reshape(D, B * LP))
        bblob = np.empty((D, BBLOB_W), ml_dtypes.bfloat16)
        bblob[:, BOFF_VW:BOFF_VW + D] = v_w[h].T.astype(ml_dtypes.bfloat16)
        bblob[:, BOFF_PW:BOFF_PW + D] = (
            proj_w[:, h * D:(h + 1) * D].T.astype(ml_dtypes.bfloat16))
        in_maps.append(dict(blob=blob, xraw=xraw, bblob=bblob))
    return in_maps


def kernel(**inputs):
    global _built_nc, last_results
    if _built_nc is None:
        _built_nc = _build()
    in_maps = _prep_in_maps(inputs)
    res = run_bass_kernel_spmd(_built_nc, in_maps, list(range(H)))
    last_results = res
    parts = np.stack([np.asarray(res.results[c]["outp"], dtype=np.float32)
                      for c in range(H)])            # [H,B,128,NT,D]
    out = parts.sum(axis=0, dtype=np.float32)        # [B, 128(p), NT(m), D]
    out = out.transpose(0, 2, 1, 3).reshape(B, L, D)  # l = m*128 + p
    out = np.ascontiguousarray(out)
    out += np.asarray(inputs["proj_b"], np.float32)[None, None, :]
    return out.astype(np.float32)
